# revision 45
# baseline (speedup 1.0000x reference)
"""Trainium2 Bass kernel for nn_DecoderPp (PointNet++-style 3-level KNN decoder).

Data-parallel over 16 graphs: core c owns graphs 2c, 2c+1; tiles of the
two graphs interleave within each level. Per 256-target tile PAIR:
- PE computes m = q.p - |p|^2/2 via a K=11 fp16 matmul: coordinates are
  hi/lo split into two fp16 halves (q.p = qh.ph + qh.pl + ql.ph, products
  exact in the f32 accumulator; dropped ql.pl ~2^-22), so the matmul runs
  at 1 cyc/col like bf16 with ~3e-5 absolute error -- 50x tighter than
  f32r (whose rounding flips neighbor selections) at 1/4 the PE cost of
  f32. Coordinate rows are host-relayout params; the p-side -|p|^2/2 fp16
  hi/lo rows are built on device (batched DVE square+reduce + PE transpose
  + row DMAs). The per-target -|q|^2/2 stays NATURAL [128, a] f32 and
  rides the PSUM->SBUF ACT staging copy as its per-partition bias -- no
  transpose, no wide single-partition-row DMA (those cost ~0.39 ns/byte
  of per-partition payload, 3.2us for a [1,4096] f32 row).
- Selection: k<=8 is one DVE max8; k=16 takes per-block max8s (5 blocks)
  whose top-8 union covers the true top-16 except ~1e-3/row (a miss only
  lowers tau -> an extra small-weight neighbor), then an exact
  top-16-of-union merge (max8 + match_replace + max8 on the 40-col tile).
  tau flows bit-exactly (max8 copies f32) so no epsilon bump is needed.
- Weights: ONE fused custom-DVE pass (registered at import into
  concourse.dve_ops) MASKED_RECIP_SUM_DECPP: Wraw = (s >= tau) ?
  recip1nr(s) : 0 with accum sw, where recip1nr = BITWISE_NOT bit-trick
  seed + one Newton step (~0.4% error, same class as the bf16 weight
  cast). Replaces the dense InstReciprocal + masked stt pair; HW-checked
  (Pool has no generic elementwise ISA and DVE has no divide ALU op, so
  this is the only single-pass form). Pool's normalize_recip ucode then
  divides by sw and casts to bf16 off the DVE/ACT critical path.
- Per-128 xbar DMA transposes feed bf16 aggregation matmuls y^T = xe^T
  W^T; consecutive same-graph tiles PAIR through agg+MLP so every ACT op
  there runs [*, 256], halving its fixed SBUF-access overhead. MLP is
  feature-major on PE, tanh/bias fused into ACT. Skips are host-relayout
  bf16 [Ck, n] params sliced per pair.
- Levels are split into pair_A (weight production, positions only) and
  pair_B (agg+MLP, needs previous level's features): level 1's first four
  pair_As pre-warm inside the level-2 stream and its pair_Bs lag two
  pairs so the kernel tail is agg/MLP-only. PSUM: s 2x2 banks + y 2 + mlp
  2 rings.
Cost-model time: 220752 ns (session start 309335, original 500480);
hw rel err ~6.4e-3.
Built on Bacc (finalize() legalizes multi-semaphore waits via EVSEM).
"""
import sys
from contextlib import ExitStack

if "/opt/trn_rl_repo" not in sys.path:
    sys.path.insert(0, "/opt/trn_rl_repo")

import ml_dtypes
import numpy as np

import concourse.bass as bass
import concourse.mybir as mybir
from concourse.bacc import Bacc
from concourse.tile import TileContext
from concourse.masks import make_identity

dt = mybir.dt
AF = mybir.ActivationFunctionType
ALU = mybir.AluOpType
AX = mybir.AxisListType

N_CORES = 8
GRAPHS_PER_CORE = 2
N3G, N2G, N1G, N0G = 64, 256, 1024, 4096  # per-graph sizes per level

NEG_BIG = -1.0e30
TAU_BUMP = 1.0 + 1.0e-6  # tau' = tau*(1+1e-6): k-th (negative) value stays selected
KROWS = 11  # 9 coord hi/lo product rows + 2 p-side sq (or ones) rows

f32 = dt.float32
f16 = dt.float16
bf16 = dt.bfloat16

USE_CUSTOM_DVE = True

# ---- fused masked-reciprocal-sum custom DVE op -------------------------
# out = (s >= tau) ? recip1nr(s) : 0 ; accum_out = sum(out). Replaces the
# dense InstReciprocal + masked scalar_tensor_tensor pair with ONE DVE
# pass. recip1nr is the BITWISE_NOT bit-trick seed (s * bitcast(~s) lands
# in [-4.5, -4] for either sign) + one Newton step: ~0.4% relative error,
# the same class as the bf16 weight cast the baseline already used. The
# building blocks (select-by-compare, BITWISE_NOT seed, add-accumulate)
# all appear in HW-validated registry ops.
_MRS_C0 = -0.23549792  # Chebyshev seed scale over [-4.5, -4]
_MRS_C1 = 2.0017324    # Newton c1

from concourse import dve_ops as _dve_ops
from concourse import dve_spec as _dve_spec


def _make_masked_recip_sum():
    name = "MASKED_RECIP_SUM_DECPP"
    if name in _dve_ops.CUSTOM_DVE_SPECS:
        return next(o for o in _dve_ops.OPS if o.name == name)
    Src0, C0, C1, C2 = (_dve_spec.Src0, _dve_spec.C0, _dve_spec.C1,
                        _dve_spec.C2)
    from operator import add as _add

    _not = _dve_spec.Bin(_dve_spec.AluOp.BITWISE_NOT, Src0, Src0)
    _y0 = _not * C1
    _y1 = _y0 * (C2 - Src0 * _y0)

    def _ref(in0, in1, s0, s1, imm2):
        x = np.ascontiguousarray(in0, dtype=np.float32)
        not_x = (~x.view(np.int32)).view(np.float32)
        y0 = not_x * s1
        y1 = (y0 * (imm2 - x * y0)).astype(np.float32)
        b = np.where(x >= s0, y1, np.float32(0.0)).astype(np.float32)
        return b, b.reshape(b.shape[0], -1).sum(axis=-1, keepdims=True)

    spec = _dve_spec.Spec(
        body=_dve_spec.select(Src0 >= C0, _y1, _dve_spec.Zero),
        accum=_add,
        accum_init=_dve_spec.Zero,
        reference=_ref,
    )
    ver = "v3"
    lowered = _dve_ops.DveOpSpec(
        name=name,
        opcode=_dve_ops._CUSTOM_DVE_ROW_BASE + len(_dve_ops.OPS),
        uops=_dve_spec.lower(spec, ver=ver),
        rd1_en=_dve_spec._has_src1(spec),
    )
    op = _dve_ops.DveOp(name, spec, False, {ver: lowered.sha(ver)})
    _dve_ops.OPS.append(op)
    _dve_ops.CUSTOM_DVE_SPECS[name] = spec
    _dve_ops._SUB_OPCODE_FOR_NAME[name] = (
        _dve_ops._CUSTOM_DVE_ROW_BASE + len(_dve_ops.OPS) - 1)
    return op


MASKED_RECIP_SUM = _make_masked_recip_sum()


def _ceil_div(a, b):
    return (a + b - 1) // b


def build_module():
    nc = Bacc()

    P = {}

    def param(name, shape, out=False, dtype=f32):
        P[name] = nc.declare_dram_parameter(name, list(shape), dtype,
                                            isOutput=out)

    param("x", (GRAPHS_PER_CORE * N3G, 256), dtype=bf16)
    param("pos", (GRAPHS_PER_CORE * N3G, 3))
    param("ps2", (GRAPHS_PER_CORE * N2G, 3))
    param("ps1", (GRAPHS_PER_CORE * N1G, 3))
    param("ps0", (GRAPHS_PER_CORE * N0G, 3))
    # host-relayout fp16 matmul operand tiles [g, 13, n]: coordinate hi/lo
    # product rows + ones rows; sq rows (9,10 on p-side / 11,12 on q-side)
    # are zero-filled by the host and written on device
    param("pT3", (GRAPHS_PER_CORE, KROWS, N3G), dtype=f16)
    param("qT3", (GRAPHS_PER_CORE, KROWS, N2G), dtype=f16)
    param("pT2", (GRAPHS_PER_CORE, KROWS, N2G), dtype=f16)
    param("qT2", (GRAPHS_PER_CORE, KROWS, N1G), dtype=f16)
    param("pT1", (GRAPHS_PER_CORE, KROWS, N1G), dtype=f16)
    param("qT1", (GRAPHS_PER_CORE, KROWS, N0G), dtype=f16)
    param("xs2T", (GRAPHS_PER_CORE, 128, N2G), dtype=bf16)
    param("xs1T", (GRAPHS_PER_CORE, 64, N1G), dtype=bf16)
    param("xs0T", (GRAPHS_PER_CORE, 3, N0G), dtype=bf16)
    for nm, shp in [
        ("W3a", (128, 384)), ("b3a", (128,)),
        ("W3b", (128, 128)), ("b3b", (128,)),
        ("W2a", (64, 192)), ("b2a", (64,)),
        ("W2b", (64, 64)), ("b2b", (64,)),
        ("W1a", (64, 67)), ("b1a", (64,)),
        ("W1b", (64, 64)), ("b1b", (64,)),
        ("W1c", (3, 64)), ("b1c", (3,)),
    ]:
        param(nm, shp)
    param("out", (GRAPHS_PER_CORE, 3, N0G), out=True)

    with TileContext(nc) as tc, ExitStack() as ctx:
        consts = ctx.enter_context(tc.tile_pool(name="consts", bufs=1))
        wpool = ctx.enter_context(tc.tile_pool(name="weights", bufs=1))
        gpool = ctx.enter_context(tc.tile_pool(name="graph", bufs=2))
        tpool = ctx.enter_context(tc.tile_pool(name="tiles", bufs=7))
        npool = ctx.enter_context(tc.tile_pool(name="narrow", bufs=12))
        # PSUM budget (8 banks x 2KB): s 2x2 banks + y 2x1 + mlp 2x1 = 8.
        # 2-deep mlp/y rings keep consecutive tiles' MLP layers from
        # serializing through a single PE<->ACT buffer.
        pspool = ctx.enter_context(tc.tile_pool(name="ps_s", bufs=2, space="PSUM"))
        psy = ctx.enter_context(tc.tile_pool(name="ps_y", bufs=2, space="PSUM"))
        psm = ctx.enter_context(tc.tile_pool(name="ps_mlp", bufs=2, space="PSUM"))

        ident0 = consts.tile([128, 128], f32)
        make_identity(nc, ident0)
        # ACT-written copy: PE transposes read this so their input waits
        # collapse onto the Activation semaphore (walrus LDW 1-wait limit)
        ident = consts.tile([128, 128], f32)
        nc.scalar.activation(ident[:, :], ident0[:, :], AF.Copy)

        # ---- weight prep: transposed chunks + f32 bias columns ----
        def prep_linear(wname, bname, O, I, splits, wdtype=bf16, q="gpsimd"):
            eng = getattr(nc, q)
            w_sb = wpool.tile([O, I], f32, tag=f"{wname}_raw")
            eng.dma_start(w_sb[:, :], P[wname].ap())
            chunks = []
            c0 = 0
            for j, cw in enumerate(splits):
                c1 = c0 + cw
                ps_t = psm.tile([128, 128], f32, tag="mlp")
                nc.tensor.transpose(ps_t[:cw, :O], w_sb[:, c0:c1],
                                    ident[:O, :O])
                wt = wpool.tile([cw, O], wdtype, tag=f"{wname}T{j}")
                nc.scalar.activation(wt[:, :], ps_t[:cw, :O], AF.Copy)
                chunks.append((wt, cw))
                c0 = c1
            bcol = wpool.tile([O, 1], f32, tag=f"{bname}col")
            eng.dma_start(bcol[:, :], P[bname].ap())
            return chunks, bcol

        def make_posT_load(dramT, g, n, tag, eng=None):
            """One [13,n] fp16 DMA loads coord hi/lo + ones rows (host
            layout); the sq chain later fills the side's sq rows. ACT DMA
            channel keeps SP free for per-tile transposes."""
            pt = gpool.tile([KROWS, n], f16, tag=tag, name=f"pt_{tag}_g{g}")
            (eng if eng is not None else nc.scalar).dma_start(
                pt[:, :], dramT.ap()[g, :, :])
            return pt

        P_HI, P_LO = 9, 10  # p-side sq row indices in the operand tiles

        def stage_sq_graph(uid, g, specs, dma_eng=None):
            """Batched -|.|^2/2 for every pos tensor of graph g.
            specs: list of (pt_or_None, dram, n, kind). One wide DVE
            square+reduce+scale for all; p-side ("p") entries get fp16
            hi/lo rows DMA'd into their operand tile (rows 9,10); q-side
            ("q") entries stay NATURAL: their per-target -|q|^2/2 columns
            ride the PSUM->SBUF staging copy as a free ACT bias.
            Returns {spec_index: (s2h_tile, col0)} for q-sides."""
            groups = []  # (pt, row0, a, n, kind)
            row0 = 0
            for pt, dram, n, kind in specs:
                a = max(1, n // 128)
                groups.append((pt, row0, a, n, kind))
                row0 += a
            atot = row0
            nball = gpool.tile([128, atot * 3], f32, tag=f"nball{uid % 2}",
                               name=f"nball_u{uid}")
            nc.vector.memset(nball[:, :], 0.0)
            for (pt, r0, a, n, kind), spec in zip(groups, specs):
                dram = spec[1]
                base = g * n
                if n >= 128:
                    src_ap = dram.ap()[base : base + n, :].rearrange(
                        "(a p) d -> p a d", p=128)
                    nc.sync.dma_start(nball[:, 3 * r0 : 3 * (r0 + a)], src_ap)
                else:
                    nc.sync.dma_start(nball[:n, 3 * r0 : 3 * r0 + 3],
                                      dram.ap()[base : base + n, :])
            sq = gpool.tile([128, atot * 3], f32, tag=f"sqall{uid % 2}",
                            name=f"sqall_u{uid}")
            nc.vector.tensor_tensor(sq[:, :], nball[:, :], nball[:, :],
                                    op=ALU.mult)
            s2 = gpool.tile([128, atot], f32, tag=f"s2all{uid % 2}",
                            name=f"s2all_u{uid}")
            nc.vector.tensor_reduce(
                s2[:, :], sq[:, :].rearrange("p (a d) -> p a d", d=3),
                axis=AX.X, op=ALU.add)
            s2h = gpool.tile([128, atot], f32, tag=f"s2hall{uid % 2}",
                             name=f"s2hall_u{uid}")
            nc.vector.tensor_scalar(s2h[:, :], s2[:, :], -0.5, None,
                                    op0=ALU.mult)
            p_atot = sum(a for _, _, a, _, kind in groups if kind == "p")
            qcols = {}
            if p_atot:
                t_ps = psm.tile([128, 128], f32, tag="mlp")
                nc.tensor.transpose(t_ps[:atot, :], s2h[:, :], ident[:, :])
                # fp16 hi/lo split of the p-side sq rows (error ~2^-22|p|^2)
                s2hiT = gpool.tile([64, 128], f16, tag=f"s2hiT{uid % 2}",
                                   name=f"s2hiT_u{uid}")
                nc.scalar.activation(s2hiT[:atot, :], t_ps[:atot, :], AF.Copy)
                s2loT = gpool.tile([64, 128], f16, tag=f"s2loT{uid % 2}",
                                   name=f"s2loT_u{uid}")
                with nc.allow_low_precision("fp16 lo residual of hi/lo "
                                            "split"):
                    nc.vector.tensor_tensor(s2loT[:atot, :], t_ps[:atot, :],
                                            s2hiT[:atot, :], op=ALU.subtract)
            eng = dma_eng if dma_eng is not None else nc.gpsimd
            for i, (pt, r0, a, n, kind) in enumerate(groups):
                if kind == "q":
                    qcols[i] = (s2h, r0)
                    continue
                if n >= 128:
                    eng.dma_start(pt[P_HI : P_HI + 1, :],
                                  s2hiT[r0 : r0 + a, :])
                    eng.dma_start(pt[P_LO : P_LO + 1, :],
                                  s2loT[r0 : r0 + a, :])
                else:
                    eng.dma_start(pt[P_HI : P_HI + 1, :],
                                  s2hiT[r0 : r0 + 1, :n])
                    eng.dma_start(pt[P_LO : P_LO + 1, :],
                                  s2loT[r0 : r0 + 1, :n])
            return qcols

        # ---------------- one interpolation+MLP level ----------------
        def make_level(lvl, ns, nt, k, Cs, xe_chunks, pTs, qTs, qsqs,
                       skipT_dram, Ck, mlp, out_tiles):
            """Returns (pair_A, pair_B, sched): pair_A(g, pi) produces the
            normalized weight transposes for one 256-target pair (needs only
            positions); pair_B(g, pi) aggregates + runs the MLP (needs the
            previous level's features). Splitting lets a later level's A
            phase pre-warm during the previous level's drain. Pairs of
            consecutive same-graph tiles share the agg/MLP stage so every
            ACT op there runs at [*, 256], halving its fixed SBUF-access
            overhead per tile. mlp: list of (chunks, bcol, tanh?, O, odt)."""

            ns_pad = max(128, ns)
            n_sch = _ceil_div(ns, 128)
            nfc = _ceil_div(Cs, 128)
            gs = list(range(GRAPHS_PER_CORE))
            ntile = nt // 128
            npair = ntile // 2
            sched = [(gs[i % len(gs)], i // len(gs))
                     for i in range(len(gs) * npair)]
            WT_store = {}

            def pair_A(g, pi):
                pT, qT = pTs[g], qTs[g]
                qsq, qc0 = qsqs[g]
                WTs = {}
                for half in range(2):
                    ti = pi * 2 + half
                    t0 = ti * 128
                    # sb = -d^2/2 : [128, ns] PSUM (K=13 fp16 matmul)
                    s_ps = pspool.tile([128, 1024], f32, tag="s")
                    qlhs = qT[:, t0 : t0 + 128]
                    for h0 in range(0, ns, 512):
                        h1 = min(ns, h0 + 512)
                        nc.tensor.matmul(s_ps[:, h0:h1], qlhs,
                                         pT[:, h0:h1],
                                         start=True, stop=True)

                    # ACT stages sb to SBUF: DVE pays the cheap SBUF access
                    # latency and Pool (no PSUM port) can read it. The
                    # per-target -|q|^2/2 term rides along as the (free)
                    # per-partition bias column -- it never touches the
                    # matmul or a row DMA.
                    s_sb = tpool.tile([128, ns_pad], f32, tag="s_sb")
                    nc.scalar.activation(s_sb[:, :ns], s_ps[:, :ns],
                                         AF.Identity,
                                         bias=qsq[:, qc0 + ti : qc0 + ti + 1])

                    # --- selection: tau = k-th largest sb per row.
                    # Per-block top-8s (5 blocks) cover the true top-16
                    # except ~1e-3/row (misses only ADD a small extra
                    # neighbor: tau from the union is <= the true k-th, so
                    # the dense compare still keeps all true neighbors);
                    # exact top-16 of the union. The k-th value flows
                    # bit-exactly (max8 copies f32) so the compare needs no
                    # epsilon bump. ---
                    if k == 16:
                        nb = 5
                        bounds = [round(i * ns / nb) for i in range(nb + 1)]
                        v8s = tpool.tile([128, 8 * nb], f32, tag="v8s")
                        for j in range(nb):
                            nc.vector.max(v8s[:, 8 * j : 8 * j + 8],
                                          s_sb[:, bounds[j] : bounds[j + 1]])
                        m16 = npool.tile([128, 16], f32, tag="m16")
                        nc.vector.max(m16[:, 0:8], v8s[:, :])
                        zapc = tpool.tile([128, 8 * nb], f32, tag="zapc")
                        nc.vector.match_replace(zapc[:, :], m16[:, 0:8],
                                                v8s[:, :], NEG_BIG)
                        nc.vector.max(m16[:, 8:16], zapc[:, :])
                        taur = m16[:, 15:16]
                    else:
                        v8 = npool.tile([128, 8], f32, tag="v8")
                        nc.vector.max(v8[:, :], s_sb[:, :ns])
                        taur = v8[:, k - 1 : k]

                    # --- weights: Wraw = (sb >= taur) * (1/sb), accum sw ---
                    Wraw = tpool.tile([128, ns_pad], f32, tag="Wraw")
                    sw = npool.tile([128, 1], f32, tag="sw")
                    if USE_CUSTOM_DVE:
                        # one fused DVE pass: bit-trick reciprocal seed + one
                        # NR step (~0.4% wrec error, same class as the bf16
                        # weight cast) + threshold select + sum accumulate
                        nc.vector._custom_dve(
                            MASKED_RECIP_SUM, out=Wraw[:, :ns],
                            in0=s_sb[:, :ns], s0=taur,
                            s1=_MRS_C0, imm2=_MRS_C1,
                            accum_out=sw[:, :])
                    else:
                        # two proven DVE passes (dense reciprocal + masked
                        # multiply-accumulate)
                        wrec = tpool.tile([128, ns_pad], bf16, tag="wrec")
                        with nc.allow_low_precision("inverse-distance "
                                                    "weights; bf16 ok"):
                            nc.vector.reciprocal(wrec[:, :ns], s_sb[:, :ns])
                        nc.vector.scalar_tensor_tensor(
                            Wraw[:, :ns], s_sb[:, :ns], taur,
                            wrec[:, :ns], op0=ALU.is_ge, op1=ALU.mult,
                            accum_out=sw[:, :])
                    # Pool ucode normalizes and casts: W = Wraw / sw -> bf16
                    W = tpool.tile([128, ns_pad], bf16, tag="W")
                    if ns < ns_pad:
                        nc.vector.memset(W[:, ns:], 0.0)
                    nc.gpsimd.normalize_recip(W[:, :ns], Wraw[:, :ns],
                                              sw[:, :])

                    # --- transpose W chunks for aggregation ---
                    WT = []
                    for j in range(ns_pad // 128):
                        wt = tpool.tile([128, 128], bf16, tag=f"WT{half}_{j}")
                        nc.sync.dma_start_transpose(
                            wt[:, :], W[:, j * 128 : (j + 1) * 128])
                        WT.append(wt)
                    WTs[half] = WT
                WT_store[(g, pi)] = WTs

            def pair_B(g, pi):
                out_tile = out_tiles[g]
                p0 = pi * 256
                WTs = WT_store.pop((g, pi))
                # --- aggregate y^T = xe^T @ W^T (both halves, [*, 256]) ---
                y_ps = []
                for fc in range(nfc):
                    f0, f1 = fc * 128, min(Cs, (fc + 1) * 128)
                    yp = psy.tile([128, 256], f32, tag="y")
                    for half in range(2):
                        c0 = half * 128
                        for j in range(n_sch):
                            kr = min(128, ns - j * 128)
                            nc.tensor.matmul(yp[: f1 - f0, c0 : c0 + 128],
                                             xe_chunks[g][j][0][:kr, f0:f1],
                                             WTs[half][j][:kr, :],
                                             start=(j == 0),
                                             stop=(j == n_sch - 1))
                    y_ps.append((yp, f1 - f0))

                # --- MLP input chunks: y^T (bf16) + skip^T ---
                in_chunks = []
                for fc, (yp, fw) in enumerate(y_ps):
                    hc = tpool.tile([128, 256], bf16, tag=f"hc{fc}")
                    nc.scalar.activation(hc[:fw, :], yp[:fw, :], AF.Copy)
                    in_chunks.append((hc, fw))
                skc = tpool.tile([Ck, 256], bf16, tag="skc")
                nc.sync.dma_start(skc[:, :],
                                  skipT_dram.ap()[g, :, p0 : p0 + 256])
                in_chunks.append((skc, Ck))

                # --- MLP (feature-major, [*, 256]) ---
                cur = in_chunks
                for li, (chunks, bcol, tanh, O, odt) in enumerate(mlp):
                    mp = psm.tile([128, 256], f32, tag="mlp")
                    nkc = len(cur)
                    for j, (ct, kr) in enumerate(cur):
                        wt, cw = chunks[j]
                        assert cw == kr, f"l{lvl} mlp{li} c{j}: {cw} != {kr}"
                        nc.tensor.matmul(mp[:O, :], wt[:, :O], ct[:kr, :],
                                         start=(j == 0), stop=(j == nkc - 1))
                    if li == len(mlp) - 1:
                        nc.scalar.activation(out_tile[:O, p0 : p0 + 256],
                                             mp[:O, :], AF.Identity,
                                             bias=bcol[:, :])
                    else:
                        ho = tpool.tile([128, 256], odt, tag=f"ho{li}")
                        nc.scalar.activation(ho[:O, :], mp[:O, :],
                                             AF.Tanh if tanh else AF.Identity,
                                             bias=bcol[:, :])
                        cur = [(ho, O)]

            return pair_A, pair_B, sched

        def prop_level(*args, pre=()):
            """Emit one level; `pre` pairs already had pair_A emitted."""
            pair_A, pair_B, sched = make_level(*args)
            for g, pi in sched:
                if (g, pi) not in pre:
                    pair_A(g, pi)
                pair_B(g, pi)

        # ---------------- per-graph pipeline ----------------
        # All pos tensors (including level 1) are loaded up front, chunked
        # and spread across the ACT/Pool DMA queues so no queue holds a
        # multi-us transfer in front of compute-critical work.
        posts = {}
        xe3s = {}
        for g in range(GRAPHS_PER_CORE):
            xe3 = gpool.tile([64, 256], bf16, tag="xe3", name=f"xe3_g{g}")
            nc.sync.dma_start(xe3[:, :], P["x"].ap()[g * 64 : (g + 1) * 64, :])
            xe3s[g] = xe3
        # levels 3+2 pos tensors (small direct [11, n] loads)
        for lvl, (pd, pn, qd, qn) in {
            3: ("pT3", N3G, "qT3", N2G),
            2: ("pT2", N2G, "qT2", N1G),
        }.items():
            for g in range(GRAPHS_PER_CORE):
                posts[(g, lvl, "p")] = make_posT_load(P[pd], g, pn,
                                                      f"pT{lvl}")
                posts[(g, lvl, "q")] = make_posT_load(P[qd], g, qn,
                                                      f"qT{lvl}")
        # level-1 pos tensors: pT1 direct; qT1 [11, 4096] split into 4
        # column chunks alternating between the Pool and ACT queues
        for g in range(GRAPHS_PER_CORE):
            posts[(g, 1, "p")] = make_posT_load(P["pT1"], g, N1G, "pT1",
                                                eng=nc.gpsimd)
            qt = gpool.tile([KROWS, N0G], f16, tag="qT1", name=f"pt_qT1_g{g}")
            for ci in range(4):
                c0, c1 = ci * 1024, (ci + 1) * 1024
                eng = nc.gpsimd if (ci + g) % 2 == 0 else nc.scalar
                eng.dma_start(qt[:, c0:c1], P["qT1"].ap()[g, :, c0:c1])
            posts[(g, 1, "q")] = qt

        # batched sq chains; q-sides stay natural for the staging bias
        qsq32 = {}
        qsq1 = {}
        for g in range(GRAPHS_PER_CORE):
            specs = [
                (posts[(g, 3, "p")], P["pos"], N3G, "p"),
                (None, P["ps2"], N2G, "q"),
                (posts[(g, 2, "p")], P["ps2"], N2G, "p"),
                (None, P["ps1"], N1G, "q"),
            ]
            qc = stage_sq_graph(g, g, specs)
            qsq32[(g, 3)] = qc[1]
            qsq32[(g, 2)] = qc[3]
        for g in range(GRAPHS_PER_CORE):
            specs = [
                (posts[(g, 1, "p")], P["ps1"], N1G, "p"),
                (None, P["ps0"], N0G, "q"),
            ]
            qc = stage_sq_graph(2 + g, g, specs)
            qsq1[g] = qc[1]

        W3aT, b3a = prep_linear("W3a", "b3a", 128, 384, [128, 128, 128],
                                q="sync")
        W3bT, b3b = prep_linear("W3b", "b3b", 128, 128, [128], q="sync")
        W2aT, b2a = prep_linear("W2a", "b2a", 64, 192, [128, 64])
        W2bT, b2b = prep_linear("W2b", "b2b", 64, 64, [64])
        W1aT, b1a = prep_linear("W1a", "b1a", 64, 67, [64, 3])
        W1bT, b1b = prep_linear("W1b", "b1b", 64, 64, [64])
        W1cT, b1c = prep_linear("W1c", "b1c", 3, 64, [64])

        h3Ts, h3nats, h2Ts, h2nats, outTs = {}, {}, {}, {}, {}
        GS = list(range(GRAPHS_PER_CORE))
        for g in GS:
            h3Ts[g] = gpool.tile([128, 256], bf16, tag="h3T", name=f"h3T_g{g}")
            h2Ts[g] = gpool.tile([64, 1024], bf16, tag="h2T", name=f"h2T_g{g}")
            outTs[g] = gpool.tile([3, 4096], f32, tag="outT", name=f"outT_g{g}")
        # h3nats/h2nats are filled AFTER their producing level; the later
        # level's pair_B only dereferences them at its own emission time.
        prop_level(3, N3G, N2G, 4, 256, {g: [(xe3s[g], 64)] for g in GS},
                   {g: posts[(g, 3, "p")] for g in GS},
                   {g: posts[(g, 3, "q")] for g in GS},
                   {g: qsq32[(g, 3)] for g in GS}, P["xs2T"], 128,
                   [(W3aT, b3a, True, 128, bf16),
                    (W3bT, b3b, False, 128, bf16)], h3Ts)
        for g in GS:
            h3nat = []
            for j in range(2):
                hn = gpool.tile([128, 128], bf16, tag=f"h3n{j}",
                                name=f"h3n{j}_g{g}")
                nc.sync.dma_start_transpose(
                    hn[:, :], h3Ts[g][:, j * 128 : (j + 1) * 128])
                h3nat.append((hn, 128))
            h3nats[g] = h3nat


        # level 1 is W-heavy on DVE: pre-warm its first pairs' weight
        # production (phase A, position-only) inside the level-2 stream so
        # DVE never idles across the level transition
        l1A, l1B, l1sched = make_level(
            1, N1G, N0G, 16, 64, h2nats,
            {g: posts[(g, 1, "p")] for g in GS},
            {g: posts[(g, 1, "q")] for g in GS},
            qsq1, P["xs0T"], 3,
            [(W1aT, b1a, True, 64, bf16),
             (W1bT, b1b, True, 64, bf16),
             (W1cT, b1c, False, 3, f32)], outTs)
        l2A, l2B, l2sched = make_level(
            2, N2G, N1G, 8, 128, h3nats,
            {g: posts[(g, 2, "p")] for g in GS},
            {g: posts[(g, 2, "q")] for g in GS},
            {g: qsq32[(g, 2)] for g in GS}, P["xs1T"], 64,
            [(W2aT, b2a, True, 64, bf16),
             (W2bT, b2b, False, 64, bf16)], h2Ts)
        l1pre = [(0, 0), (1, 0), (0, 1), (1, 1)]
        pre_at = {0: 0, 1: 1, 2: 2, 3: 3}
        for i, (g, pi) in enumerate(l2sched):
            l2A(g, pi)
            l2B(g, pi)
            if i in pre_at:
                l1A(*l1pre[pre_at[i]])
        for g in GS:
            h2nat = []
            for j in range(8):
                hn = gpool.tile([128, 64], bf16, tag=f"h2n{j}",
                                name=f"h2n{j}_g{g}")
                nc.sync.dma_start_transpose(
                    hn[:, :], h2Ts[g][:, j * 128 : (j + 1) * 128])
                h2nat.append((hn, 128))
            h2nats[g] = h2nat

        # B lags A so the kernel tail is agg/MLP-only (no trailing dense
        # DVE chain after the last weight production)
        lagq = []
        for g, pi in l1sched:
            if (g, pi) not in l1pre:
                l1A(g, pi)
            lagq.append((g, pi))
            if len(lagq) > 2:
                l1B(*lagq.pop(0))
        for e in lagq:
            l1B(*e)
        for g in GS:
            for qi in range(8):
                c0, c1 = qi * 512, (qi + 1) * 512
                eng = nc.sync if (g + qi) % 2 == 0 else nc.scalar
                eng.dma_start(P["out"].ap()[g, :, c0:c1],
                              outTs[g][:, c0:c1])

    return nc, P


_NC = None


def _get_nc():
    global _NC
    if _NC is None:
        nc = build_module()[0]
        nc.finalize()  # Bacc lowering: EVSEM wait legalization + reg alloc
        _NC = nc
    return _NC


def _hilo(a):
    """fp16 hi/lo split: a ~= hi + lo with products of halves exact in f32."""
    hi = a.astype(np.float16)
    lo = (a - hi.astype(np.float32)).astype(np.float16)
    return hi, lo


def _pack_side(pos, ng, is_q):
    """[2*ng, 3] f32 -> [2, 11, ng] fp16 matmul operand rows.

    p-side rows: [pxh pxl pxh  pyh pyl pyh  pzh pzl pzh  0 0]
      (rows 9,10 = -|p|^2/2 hi/lo, written on device)
    q-side rows: [qxh qxh qxl  qyh qyh qyl  qzh qzh qzl  1 1]
      (the -|q|^2/2 term is added per-partition at PSUM->SBUF staging)
    """
    c = pos.reshape(2, ng, 3).transpose(0, 2, 1)  # [2, 3, ng]
    hi, lo = _hilo(c)
    t = np.zeros((2, KROWS, ng), np.float16)
    for d in range(3):
        if is_q:
            t[:, 3 * d + 0] = hi[:, d]
            t[:, 3 * d + 1] = hi[:, d]
            t[:, 3 * d + 2] = lo[:, d]
        else:
            t[:, 3 * d + 0] = hi[:, d]
            t[:, 3 * d + 1] = lo[:, d]
            t[:, 3 * d + 2] = hi[:, d]
    if is_q:
        t[:, 9] = 1.0
        t[:, 10] = 1.0
    return np.ascontiguousarray(t)


def shard_inputs(inputs):
    f = lambda name: np.ascontiguousarray(np.asarray(inputs[name], np.float32))
    pos_arrs = {
        "pos": (f("pos"), N3G), "ps2": (f("pos_skip2"), N2G),
        "ps1": (f("pos_skip1"), N1G), "ps0": (f("pos_skip0"), N0G),
    }
    skips = {
        "xs2T": (f("x_skip2"), N2G), "xs1T": (f("x_skip1"), N1G),
        "xs0T": (f("x_skip0"), N0G),
    }
    weights = {k: f(k) for k in ["W3a", "b3a", "W3b", "b3b", "W2a", "b2a",
                                 "W2b", "b2b", "W1a", "b1a", "W1b", "b1b",
                                 "W1c", "b1c"]}
    # (param, source, is_q)
    sides = [("pT3", "pos", False), ("qT3", "ps2", True),
             ("pT2", "ps2", False), ("qT2", "ps1", True),
             ("pT1", "ps1", False), ("qT1", "ps0", True)]
    x_full = f("x")
    in_maps = []
    for c in range(N_CORES):
        m = dict(weights)
        sub_pos = {}
        for nm, (arr, ng) in pos_arrs.items():
            sub = np.ascontiguousarray(arr[2 * c * ng : (2 * c + 2) * ng])
            m[nm] = sub
            sub_pos[nm] = (sub, ng)
        for pnm, src, is_q in sides:
            sub, ng = sub_pos[src]
            m[pnm] = _pack_side(sub, ng, is_q)
        for nm, (arr, ng) in skips.items():
            sub = arr[2 * c * ng : (2 * c + 2) * ng]
            d = sub.shape[1]
            t = sub.reshape(2, ng, d).transpose(0, 2, 1)
            m[nm] = np.ascontiguousarray(t.astype(ml_dtypes.bfloat16))
        m["x"] = np.ascontiguousarray(
            x_full[2 * c * N3G : (2 * c + 2) * N3G].astype(ml_dtypes.bfloat16))
        in_maps.append(m)
    return in_maps


def kernel(**inputs):
    nc = _get_nc()
    in_maps = shard_inputs(inputs)
    from concourse.bass_utils import run_bass_kernel_spmd

    res = run_bass_kernel_spmd(nc, in_maps, list(range(N_CORES)))
    # device writes [g, 3, n]; restore the [n_total, 3] layout
    return np.concatenate(
        [np.asarray(r["out"], np.float32).transpose(0, 2, 1).reshape(-1, 3)
         for r in res.results], axis=0)


if __name__ == "__main__":
    nc, _ = build_module()
    print("build ok")


# revision 46
# speedup vs baseline: 1.0081x; 1.0081x over previous
"""Trainium2 Bass kernel for nn_DecoderPp (PointNet++-style 3-level KNN decoder).

Data-parallel over 16 graphs: core c owns graphs 2c, 2c+1; tiles of the
two graphs interleave within each level. Per 256-target tile PAIR:
- PE computes m = q.p - |p|^2/2 via a K=11 fp16 matmul: coordinates are
  hi/lo split into two fp16 halves (q.p = qh.ph + qh.pl + ql.ph, products
  exact in the f32 accumulator; dropped ql.pl ~2^-22), so the matmul runs
  at 1 cyc/col like bf16 with ~3e-5 absolute error -- 50x tighter than
  f32r (whose rounding flips neighbor selections) at 1/4 the PE cost of
  f32. Coordinate rows are host-relayout params; the p-side -|p|^2/2 fp16
  hi/lo rows are built on device (batched DVE square+reduce + PE transpose
  + row DMAs). The per-target -|q|^2/2 stays NATURAL [128, a] f32 and
  rides the PSUM->SBUF ACT staging copy as its per-partition bias -- no
  transpose, no wide single-partition-row DMA (those cost ~0.39 ns/byte
  of per-partition payload, 3.2us for a [1,4096] f32 row).
- Selection: k<=8 is one DVE max8; k=16 takes per-block max8s (5 blocks)
  whose top-8 union covers the true top-16 except ~1e-3/row (a miss only
  lowers tau -> an extra small-weight neighbor), then an exact
  top-16-of-union merge (max8 + match_replace + max8 on the 40-col tile).
  tau flows bit-exactly (max8 copies f32) so no epsilon bump is needed.
- Weights: ONE fused custom-DVE pass (registered at import into
  concourse.dve_ops) MASKED_RECIP_SUM_DECPP: Wraw = (s >= tau) ?
  recip1nr(s) : 0 with accum sw, where recip1nr = BITWISE_NOT bit-trick
  seed + one Newton step (~0.4% error, same class as the bf16 weight
  cast). Replaces the dense InstReciprocal + masked stt pair; HW-checked
  (Pool has no generic elementwise ISA and DVE has no divide ALU op, so
  this is the only single-pass form). Pool's normalize_recip ucode then
  divides by sw and casts to bf16 off the DVE/ACT critical path.
- Per-128 xbar DMA transposes feed bf16 aggregation matmuls y^T = xe^T
  W^T; consecutive same-graph tiles PAIR through agg+MLP so every ACT op
  there runs [*, 256], halving its fixed SBUF-access overhead. MLP is
  feature-major on PE, tanh/bias fused into ACT. Skips are host-relayout
  bf16 [Ck, n] params sliced per pair.
- Levels are split into pair_A (weight production, positions only) and
  pair_B (agg+MLP, needs previous level's features): level 1's first four
  pair_As pre-warm inside the level-2 stream and its pair_Bs lag two
  pairs so the kernel tail is agg/MLP-only. PSUM: s 2x2 banks + y 2 + mlp
  2 rings.
Cost-model time: 220752 ns (session start 309335, original 500480);
hw rel err ~6.4e-3.
Built on Bacc (finalize() legalizes multi-semaphore waits via EVSEM).
"""
import sys
from contextlib import ExitStack

if "/opt/trn_rl_repo" not in sys.path:
    sys.path.insert(0, "/opt/trn_rl_repo")

import ml_dtypes
import numpy as np

import concourse.bass as bass
import concourse.mybir as mybir
from concourse.bacc import Bacc
from concourse.tile import TileContext
from concourse.masks import make_identity

dt = mybir.dt
AF = mybir.ActivationFunctionType
ALU = mybir.AluOpType
AX = mybir.AxisListType

N_CORES = 8
GRAPHS_PER_CORE = 2
N3G, N2G, N1G, N0G = 64, 256, 1024, 4096  # per-graph sizes per level

NEG_BIG = -1.0e30
TAU_BUMP = 1.0 + 1.0e-6  # tau' = tau*(1+1e-6): k-th (negative) value stays selected
KROWS = 11  # 9 coord hi/lo product rows + 2 p-side sq (or ones) rows

f32 = dt.float32
f16 = dt.float16
bf16 = dt.bfloat16

USE_CUSTOM_DVE = True

# ---- fused masked-reciprocal-sum custom DVE op -------------------------
# out = (s >= tau) ? recip1nr(s) : 0 ; accum_out = sum(out). Replaces the
# dense InstReciprocal + masked scalar_tensor_tensor pair with ONE DVE
# pass. recip1nr is the BITWISE_NOT bit-trick seed (s * bitcast(~s) lands
# in [-4.5, -4] for either sign) + one Newton step: ~0.4% relative error,
# the same class as the bf16 weight cast the baseline already used. The
# building blocks (select-by-compare, BITWISE_NOT seed, add-accumulate)
# all appear in HW-validated registry ops.
_MRS_C0 = -0.23549792  # Chebyshev seed scale over [-4.5, -4]
_MRS_C1 = 2.0017324    # Newton c1

from concourse import dve_ops as _dve_ops
from concourse import dve_spec as _dve_spec


def _make_masked_recip_sum():
    name = "MASKED_RECIP_SUM_DECPP"
    if name in _dve_ops.CUSTOM_DVE_SPECS:
        return next(o for o in _dve_ops.OPS if o.name == name)
    Src0, C0, C1, C2 = (_dve_spec.Src0, _dve_spec.C0, _dve_spec.C1,
                        _dve_spec.C2)
    from operator import add as _add

    _not = _dve_spec.Bin(_dve_spec.AluOp.BITWISE_NOT, Src0, Src0)
    _y0 = _not * C1
    _y1 = _y0 * (C2 - Src0 * _y0)

    def _ref(in0, in1, s0, s1, imm2):
        x = np.ascontiguousarray(in0, dtype=np.float32)
        not_x = (~x.view(np.int32)).view(np.float32)
        y0 = not_x * s1
        y1 = (y0 * (imm2 - x * y0)).astype(np.float32)
        b = np.where(x >= s0, y1, np.float32(0.0)).astype(np.float32)
        return b, b.reshape(b.shape[0], -1).sum(axis=-1, keepdims=True)

    spec = _dve_spec.Spec(
        body=_dve_spec.select(Src0 >= C0, _y1, _dve_spec.Zero),
        accum=_add,
        accum_init=_dve_spec.Zero,
        reference=_ref,
    )
    ver = "v3"
    lowered = _dve_ops.DveOpSpec(
        name=name,
        opcode=_dve_ops._CUSTOM_DVE_ROW_BASE + len(_dve_ops.OPS),
        uops=_dve_spec.lower(spec, ver=ver),
        rd1_en=_dve_spec._has_src1(spec),
    )
    op = _dve_ops.DveOp(name, spec, False, {ver: lowered.sha(ver)})
    _dve_ops.OPS.append(op)
    _dve_ops.CUSTOM_DVE_SPECS[name] = spec
    _dve_ops._SUB_OPCODE_FOR_NAME[name] = (
        _dve_ops._CUSTOM_DVE_ROW_BASE + len(_dve_ops.OPS) - 1)
    return op


MASKED_RECIP_SUM = _make_masked_recip_sum()


def _ceil_div(a, b):
    return (a + b - 1) // b


def build_module():
    nc = Bacc()

    P = {}

    def param(name, shape, out=False, dtype=f32):
        P[name] = nc.declare_dram_parameter(name, list(shape), dtype,
                                            isOutput=out)

    param("x", (GRAPHS_PER_CORE * N3G, 256), dtype=bf16)
    param("pos", (GRAPHS_PER_CORE * N3G, 3))
    param("ps2", (GRAPHS_PER_CORE * N2G, 3))
    param("ps1", (GRAPHS_PER_CORE * N1G, 3))
    param("ps0", (GRAPHS_PER_CORE * N0G, 3))
    # host-relayout fp16 matmul operand tiles [g, 13, n]: coordinate hi/lo
    # product rows + ones rows; sq rows (9,10 on p-side / 11,12 on q-side)
    # are zero-filled by the host and written on device
    param("pT3", (GRAPHS_PER_CORE, KROWS, N3G), dtype=f16)
    param("qT3", (GRAPHS_PER_CORE, KROWS, N2G), dtype=f16)
    param("pT2", (GRAPHS_PER_CORE, KROWS, N2G), dtype=f16)
    param("qT2", (GRAPHS_PER_CORE, KROWS, N1G), dtype=f16)
    param("pT1", (GRAPHS_PER_CORE, KROWS, N1G), dtype=f16)
    param("qT1", (GRAPHS_PER_CORE, KROWS, N0G), dtype=f16)
    param("xs2T", (GRAPHS_PER_CORE, 128, N2G), dtype=bf16)
    param("xs1T", (GRAPHS_PER_CORE, 64, N1G), dtype=bf16)
    param("xs0T", (GRAPHS_PER_CORE, 3, N0G), dtype=bf16)
    for nm, shp in [
        ("W3a", (128, 384)), ("b3a", (128,)),
        ("W3b", (128, 128)), ("b3b", (128,)),
        ("W2a", (64, 192)), ("b2a", (64,)),
        ("W2b", (64, 64)), ("b2b", (64,)),
        ("W1a", (64, 67)), ("b1a", (64,)),
        ("W1b", (64, 64)), ("b1b", (64,)),
        ("W1c", (3, 64)), ("b1c", (3,)),
    ]:
        param(nm, shp)
    param("out", (GRAPHS_PER_CORE, 3, N0G), out=True)

    with TileContext(nc) as tc, ExitStack() as ctx:
        consts = ctx.enter_context(tc.tile_pool(name="consts", bufs=1))
        wpool = ctx.enter_context(tc.tile_pool(name="weights", bufs=1))
        gpool = ctx.enter_context(tc.tile_pool(name="graph", bufs=2))
        tpool = ctx.enter_context(tc.tile_pool(name="tiles", bufs=7))
        npool = ctx.enter_context(tc.tile_pool(name="narrow", bufs=12))
        # PSUM budget (8 banks x 2KB): s 2x2 banks + y 2x1 + mlp 2x1 = 8.
        # 2-deep mlp/y rings keep consecutive tiles' MLP layers from
        # serializing through a single PE<->ACT buffer.
        pspool = ctx.enter_context(tc.tile_pool(name="ps_s", bufs=2, space="PSUM"))
        psy = ctx.enter_context(tc.tile_pool(name="ps_y", bufs=2, space="PSUM"))
        psm = ctx.enter_context(tc.tile_pool(name="ps_mlp", bufs=2, space="PSUM"))

        ident0 = consts.tile([128, 128], f32)
        make_identity(nc, ident0)
        # ACT-written copy: PE transposes read this so their input waits
        # collapse onto the Activation semaphore (walrus LDW 1-wait limit)
        ident = consts.tile([128, 128], f32)
        nc.scalar.activation(ident[:, :], ident0[:, :], AF.Copy)

        # ---- weight prep: transposed chunks + f32 bias columns ----
        def prep_linear(wname, bname, O, I, splits, wdtype=bf16, q="gpsimd"):
            eng = getattr(nc, q)
            w_sb = wpool.tile([O, I], f32, tag=f"{wname}_raw")
            eng.dma_start(w_sb[:, :], P[wname].ap())
            chunks = []
            c0 = 0
            for j, cw in enumerate(splits):
                c1 = c0 + cw
                ps_t = psm.tile([128, 128], f32, tag="mlp")
                nc.tensor.transpose(ps_t[:cw, :O], w_sb[:, c0:c1],
                                    ident[:O, :O])
                wt = wpool.tile([cw, O], wdtype, tag=f"{wname}T{j}")
                nc.scalar.activation(wt[:, :], ps_t[:cw, :O], AF.Copy)
                chunks.append((wt, cw))
                c0 = c1
            bcol = wpool.tile([O, 1], f32, tag=f"{bname}col")
            eng.dma_start(bcol[:, :], P[bname].ap())
            return chunks, bcol

        def make_posT_load(dramT, g, n, tag, eng=None):
            """One [13,n] fp16 DMA loads coord hi/lo + ones rows (host
            layout); the sq chain later fills the side's sq rows. ACT DMA
            channel keeps SP free for per-tile transposes."""
            pt = gpool.tile([KROWS, n], f16, tag=tag, name=f"pt_{tag}_g{g}")
            (eng if eng is not None else nc.scalar).dma_start(
                pt[:, :], dramT.ap()[g, :, :])
            return pt

        P_HI, P_LO = 9, 10  # p-side sq row indices in the operand tiles

        def stage_sq_graph(uid, g, specs, dma_eng=None):
            """Batched -|.|^2/2 for every pos tensor of graph g.
            specs: list of (pt_or_None, dram, n, kind). One wide DVE
            square+reduce+scale for all; p-side ("p") entries get fp16
            hi/lo rows DMA'd into their operand tile (rows 9,10); q-side
            ("q") entries stay NATURAL: their per-target -|q|^2/2 columns
            ride the PSUM->SBUF staging copy as a free ACT bias.
            Returns {spec_index: (s2h_tile, col0)} for q-sides."""
            groups = []  # (pt, row0, a, n, kind)
            row0 = 0
            for pt, dram, n, kind in specs:
                a = max(1, n // 128)
                groups.append((pt, row0, a, n, kind))
                row0 += a
            atot = row0
            nball = gpool.tile([128, atot * 3], f32, tag=f"nball{uid % 2}",
                               name=f"nball_u{uid}")
            nc.vector.memset(nball[:, :], 0.0)
            for (pt, r0, a, n, kind), spec in zip(groups, specs):
                dram = spec[1]
                base = g * n
                if n >= 128:
                    src_ap = dram.ap()[base : base + n, :].rearrange(
                        "(a p) d -> p a d", p=128)
                    nc.sync.dma_start(nball[:, 3 * r0 : 3 * (r0 + a)], src_ap)
                else:
                    nc.sync.dma_start(nball[:n, 3 * r0 : 3 * r0 + 3],
                                      dram.ap()[base : base + n, :])
            sq = gpool.tile([128, atot * 3], f32, tag=f"sqall{uid % 2}",
                            name=f"sqall_u{uid}")
            nc.vector.tensor_tensor(sq[:, :], nball[:, :], nball[:, :],
                                    op=ALU.mult)
            s2 = gpool.tile([128, atot], f32, tag=f"s2all{uid % 2}",
                            name=f"s2all_u{uid}")
            nc.vector.tensor_reduce(
                s2[:, :], sq[:, :].rearrange("p (a d) -> p a d", d=3),
                axis=AX.X, op=ALU.add)
            s2h = gpool.tile([128, atot], f32, tag=f"s2hall{uid % 2}",
                             name=f"s2hall_u{uid}")
            nc.vector.tensor_scalar(s2h[:, :], s2[:, :], -0.5, None,
                                    op0=ALU.mult)
            p_atot = sum(a for _, _, a, _, kind in groups if kind == "p")
            qcols = {}
            if p_atot:
                t_ps = psm.tile([128, 128], f32, tag="mlp")
                nc.tensor.transpose(t_ps[:atot, :], s2h[:, :], ident[:, :])
                # fp16 hi/lo split of the p-side sq rows (error ~2^-22|p|^2)
                s2hiT = gpool.tile([64, 128], f16, tag=f"s2hiT{uid % 2}",
                                   name=f"s2hiT_u{uid}")
                nc.scalar.activation(s2hiT[:atot, :], t_ps[:atot, :], AF.Copy)
                s2loT = gpool.tile([64, 128], f16, tag=f"s2loT{uid % 2}",
                                   name=f"s2loT_u{uid}")
                with nc.allow_low_precision("fp16 lo residual of hi/lo "
                                            "split"):
                    nc.vector.tensor_tensor(s2loT[:atot, :], t_ps[:atot, :],
                                            s2hiT[:atot, :], op=ALU.subtract)
            eng = dma_eng if dma_eng is not None else nc.gpsimd
            for i, (pt, r0, a, n, kind) in enumerate(groups):
                if kind == "q":
                    qcols[i] = (s2h, r0)
                    continue
                if n >= 128:
                    eng.dma_start(pt[P_HI : P_HI + 1, :],
                                  s2hiT[r0 : r0 + a, :])
                    eng.dma_start(pt[P_LO : P_LO + 1, :],
                                  s2loT[r0 : r0 + a, :])
                else:
                    eng.dma_start(pt[P_HI : P_HI + 1, :],
                                  s2hiT[r0 : r0 + 1, :n])
                    eng.dma_start(pt[P_LO : P_LO + 1, :],
                                  s2loT[r0 : r0 + 1, :n])
            return qcols

        # ---------------- one interpolation+MLP level ----------------
        def make_level(lvl, ns, nt, k, Cs, xe_chunks, pTs, qTs, qsqs,
                       skipT_dram, Ck, mlp, out_tiles):
            """Returns (pair_A, pair_B, sched): pair_A(g, pi) produces the
            normalized weight transposes for one 256-target pair (needs only
            positions); pair_B(g, pi) aggregates + runs the MLP (needs the
            previous level's features). Splitting lets a later level's A
            phase pre-warm during the previous level's drain. Pairs of
            consecutive same-graph tiles share the agg/MLP stage so every
            ACT op there runs at [*, 256], halving its fixed SBUF-access
            overhead per tile. mlp: list of (chunks, bcol, tanh?, O, odt)."""

            ns_pad = max(128, ns)
            n_sch = _ceil_div(ns, 128)
            nfc = _ceil_div(Cs, 128)
            gs = list(range(GRAPHS_PER_CORE))
            ntile = nt // 128
            npair = ntile // 2
            sched = [(gs[i % len(gs)], i // len(gs))
                     for i in range(len(gs) * npair)]
            WT_store = {}

            def pair_A(g, pi):
                pT, qT = pTs[g], qTs[g]
                qsq, qc0 = qsqs[g]
                WTs = {}
                for half in range(2):
                    ti = pi * 2 + half
                    t0 = ti * 128
                    # sb = -d^2/2 : [128, ns] PSUM (K=13 fp16 matmul)
                    s_ps = pspool.tile([128, 1024], f32, tag="s")
                    qlhs = qT[:, t0 : t0 + 128]
                    for h0 in range(0, ns, 512):
                        h1 = min(ns, h0 + 512)
                        nc.tensor.matmul(s_ps[:, h0:h1], qlhs,
                                         pT[:, h0:h1],
                                         start=True, stop=True)

                    # ACT stages sb to SBUF: DVE pays the cheap SBUF access
                    # latency and Pool (no PSUM port) can read it. The
                    # per-target -|q|^2/2 term rides along as the (free)
                    # per-partition bias column -- it never touches the
                    # matmul or a row DMA.
                    s_sb = tpool.tile([128, ns_pad], f32, tag="s_sb")
                    nc.scalar.activation(s_sb[:, :ns], s_ps[:, :ns],
                                         AF.Identity,
                                         bias=qsq[:, qc0 + ti : qc0 + ti + 1])

                    # --- selection: tau = k-th largest sb per row.
                    # Per-block top-8s (5 blocks) cover the true top-16
                    # except ~1e-3/row (misses only ADD a small extra
                    # neighbor: tau from the union is <= the true k-th, so
                    # the dense compare still keeps all true neighbors);
                    # exact top-16 of the union. The k-th value flows
                    # bit-exactly (max8 copies f32) so the compare needs no
                    # epsilon bump. ---
                    if k == 16:
                        nb = 5
                        bounds = [round(i * ns / nb) for i in range(nb + 1)]
                        v8s = tpool.tile([128, 8 * nb], f32, tag="v8s")
                        for j in range(nb):
                            nc.vector.max(v8s[:, 8 * j : 8 * j + 8],
                                          s_sb[:, bounds[j] : bounds[j + 1]])
                        m16 = npool.tile([128, 16], f32, tag="m16")
                        nc.vector.max(m16[:, 0:8], v8s[:, :])
                        zapc = tpool.tile([128, 8 * nb], f32, tag="zapc")
                        nc.vector.match_replace(zapc[:, :], m16[:, 0:8],
                                                v8s[:, :], NEG_BIG)
                        nc.vector.max(m16[:, 8:16], zapc[:, :])
                        taur = m16[:, 15:16]
                    else:
                        v8 = npool.tile([128, 8], f32, tag="v8")
                        nc.vector.max(v8[:, :], s_sb[:, :ns])
                        taur = v8[:, k - 1 : k]

                    # --- weights: Wraw = (sb >= taur) * (1/sb), accum sw ---
                    Wraw = tpool.tile([128, ns_pad], f32, tag="Wraw")
                    sw = npool.tile([128, 1], f32, tag="sw")
                    if USE_CUSTOM_DVE:
                        # one fused DVE pass: bit-trick reciprocal seed + one
                        # NR step (~0.4% wrec error, same class as the bf16
                        # weight cast) + threshold select + sum accumulate
                        nc.vector._custom_dve(
                            MASKED_RECIP_SUM, out=Wraw[:, :ns],
                            in0=s_sb[:, :ns], s0=taur,
                            s1=_MRS_C0, imm2=_MRS_C1,
                            accum_out=sw[:, :])
                    else:
                        # two proven DVE passes (dense reciprocal + masked
                        # multiply-accumulate)
                        wrec = tpool.tile([128, ns_pad], bf16, tag="wrec")
                        with nc.allow_low_precision("inverse-distance "
                                                    "weights; bf16 ok"):
                            nc.vector.reciprocal(wrec[:, :ns], s_sb[:, :ns])
                        nc.vector.scalar_tensor_tensor(
                            Wraw[:, :ns], s_sb[:, :ns], taur,
                            wrec[:, :ns], op0=ALU.is_ge, op1=ALU.mult,
                            accum_out=sw[:, :])
                    # Pool ucode normalizes and casts: W = Wraw / sw -> bf16
                    W = tpool.tile([128, ns_pad], bf16, tag="W")
                    if ns < ns_pad:
                        nc.vector.memset(W[:, ns:], 0.0)
                    nc.gpsimd.normalize_recip(W[:, :ns], Wraw[:, :ns],
                                              sw[:, :])

                    # --- transpose W chunks for aggregation ---
                    WT = []
                    for j in range(ns_pad // 128):
                        wt = tpool.tile([128, 128], bf16, tag=f"WT{half}_{j}")
                        nc.sync.dma_start_transpose(
                            wt[:, :], W[:, j * 128 : (j + 1) * 128])
                        WT.append(wt)
                    WTs[half] = WT
                WT_store[(g, pi)] = WTs

            def pair_B(g, pi):
                out_tile = out_tiles[g]
                p0 = pi * 256
                WTs = WT_store.pop((g, pi))
                # --- aggregate y^T = xe^T @ W^T (both halves, [*, 256]) ---
                y_ps = []
                for fc in range(nfc):
                    f0, f1 = fc * 128, min(Cs, (fc + 1) * 128)
                    yp = psy.tile([128, 256], f32, tag="y")
                    for half in range(2):
                        c0 = half * 128
                        for j in range(n_sch):
                            kr = min(128, ns - j * 128)
                            nc.tensor.matmul(yp[: f1 - f0, c0 : c0 + 128],
                                             xe_chunks[g][j][0][:kr, f0:f1],
                                             WTs[half][j][:kr, :],
                                             start=(j == 0),
                                             stop=(j == n_sch - 1))
                    y_ps.append((yp, f1 - f0))

                # --- MLP input chunks: y^T (bf16) + skip^T ---
                in_chunks = []
                for fc, (yp, fw) in enumerate(y_ps):
                    hc = tpool.tile([128, 256], bf16, tag=f"hc{fc}")
                    nc.scalar.activation(hc[:fw, :], yp[:fw, :], AF.Copy)
                    in_chunks.append((hc, fw))
                skc = tpool.tile([Ck, 256], bf16, tag="skc")
                nc.sync.dma_start(skc[:, :],
                                  skipT_dram.ap()[g, :, p0 : p0 + 256])
                in_chunks.append((skc, Ck))

                # --- MLP (feature-major, [*, 256]) ---
                cur = in_chunks
                for li, (chunks, bcol, tanh, O, odt) in enumerate(mlp):
                    mp = psm.tile([128, 256], f32, tag="mlp")
                    nkc = len(cur)
                    for j, (ct, kr) in enumerate(cur):
                        wt, cw = chunks[j]
                        assert cw == kr, f"l{lvl} mlp{li} c{j}: {cw} != {kr}"
                        nc.tensor.matmul(mp[:O, :], wt[:, :O], ct[:kr, :],
                                         start=(j == 0), stop=(j == nkc - 1))
                    if li == len(mlp) - 1:
                        nc.scalar.activation(out_tile[:O, p0 : p0 + 256],
                                             mp[:O, :], AF.Identity,
                                             bias=bcol[:, :])
                    else:
                        ho = tpool.tile([128, 256], odt, tag=f"ho{li}")
                        nc.scalar.activation(ho[:O, :], mp[:O, :],
                                             AF.Tanh if tanh else AF.Identity,
                                             bias=bcol[:, :])
                        cur = [(ho, O)]

            return pair_A, pair_B, sched

        def prop_level(*args, pre=()):
            """Emit one level; `pre` pairs already had pair_A emitted."""
            pair_A, pair_B, sched = make_level(*args)
            for g, pi in sched:
                if (g, pi) not in pre:
                    pair_A(g, pi)
                pair_B(g, pi)

        # ---------------- per-graph pipeline ----------------
        # All pos tensors (including level 1) are loaded up front, chunked
        # and spread across the ACT/Pool DMA queues so no queue holds a
        # multi-us transfer in front of compute-critical work.
        posts = {}
        xe3s = {}
        for g in range(GRAPHS_PER_CORE):
            xe3 = gpool.tile([64, 256], bf16, tag="xe3", name=f"xe3_g{g}")
            nc.sync.dma_start(xe3[:, :], P["x"].ap()[g * 64 : (g + 1) * 64, :])
            xe3s[g] = xe3
        # levels 3+2 pos tensors (small direct [11, n] loads)
        for lvl, (pd, pn, qd, qn) in {
            3: ("pT3", N3G, "qT3", N2G),
            2: ("pT2", N2G, "qT2", N1G),
        }.items():
            for g in range(GRAPHS_PER_CORE):
                posts[(g, lvl, "p")] = make_posT_load(P[pd], g, pn,
                                                      f"pT{lvl}")
                posts[(g, lvl, "q")] = make_posT_load(P[qd], g, qn,
                                                      f"qT{lvl}")
        # level-1 pos tensors: pT1 direct; qT1 [11, 4096] split into 4
        # column chunks alternating between the Pool and ACT queues
        for g in range(GRAPHS_PER_CORE):
            posts[(g, 1, "p")] = make_posT_load(P["pT1"], g, N1G, "pT1",
                                                eng=nc.gpsimd)
            qt = gpool.tile([KROWS, N0G], f16, tag="qT1", name=f"pt_qT1_g{g}")
            for ci in range(4):
                c0, c1 = ci * 1024, (ci + 1) * 1024
                eng = nc.gpsimd if (ci + g) % 2 == 0 else nc.scalar
                eng.dma_start(qt[:, c0:c1], P["qT1"].ap()[g, :, c0:c1])
            posts[(g, 1, "q")] = qt

        # batched sq chains; q-sides stay natural for the staging bias
        qsq32 = {}
        qsq1 = {}
        for g in range(GRAPHS_PER_CORE):
            specs = [
                (posts[(g, 3, "p")], P["pos"], N3G, "p"),
                (None, P["ps2"], N2G, "q"),
                (posts[(g, 2, "p")], P["ps2"], N2G, "p"),
                (None, P["ps1"], N1G, "q"),
            ]
            qc = stage_sq_graph(g, g, specs)
            qsq32[(g, 3)] = qc[1]
            qsq32[(g, 2)] = qc[3]
        for g in range(GRAPHS_PER_CORE):
            specs = [
                (posts[(g, 1, "p")], P["ps1"], N1G, "p"),
                (None, P["ps0"], N0G, "q"),
            ]
            qc = stage_sq_graph(2 + g, g, specs)
            qsq1[g] = qc[1]

        W3aT, b3a = prep_linear("W3a", "b3a", 128, 384, [128, 128, 128],
                                q="sync")
        W3bT, b3b = prep_linear("W3b", "b3b", 128, 128, [128], q="sync")
        W2aT, b2a = prep_linear("W2a", "b2a", 64, 192, [128, 64])
        W2bT, b2b = prep_linear("W2b", "b2b", 64, 64, [64])
        W1aT, b1a = prep_linear("W1a", "b1a", 64, 67, [64, 3])
        W1bT, b1b = prep_linear("W1b", "b1b", 64, 64, [64])
        W1cT, b1c = prep_linear("W1c", "b1c", 3, 64, [64])

        h3Ts, h3nats, h2Ts, h2nats, outTs = {}, {}, {}, {}, {}
        GS = list(range(GRAPHS_PER_CORE))
        for g in GS:
            h3Ts[g] = gpool.tile([128, 256], bf16, tag="h3T", name=f"h3T_g{g}")
            h2Ts[g] = gpool.tile([64, 1024], bf16, tag="h2T", name=f"h2T_g{g}")
            outTs[g] = gpool.tile([3, 4096], f32, tag="outT", name=f"outT_g{g}")
        # h3nats/h2nats are filled AFTER their producing level; the later
        # level's pair_B only dereferences them at its own emission time.
        prop_level(3, N3G, N2G, 4, 256, {g: [(xe3s[g], 64)] for g in GS},
                   {g: posts[(g, 3, "p")] for g in GS},
                   {g: posts[(g, 3, "q")] for g in GS},
                   {g: qsq32[(g, 3)] for g in GS}, P["xs2T"], 128,
                   [(W3aT, b3a, True, 128, bf16),
                    (W3bT, b3b, False, 128, bf16)], h3Ts)
        for g in GS:
            h3nat = []
            for j in range(2):
                hn = gpool.tile([128, 128], bf16, tag=f"h3n{j}",
                                name=f"h3n{j}_g{g}")
                nc.sync.dma_start_transpose(
                    hn[:, :], h3Ts[g][:, j * 128 : (j + 1) * 128])
                h3nat.append((hn, 128))
            h3nats[g] = h3nat


        # level 1 is W-heavy on DVE: pre-warm its first pairs' weight
        # production (phase A, position-only) inside the level-2 stream so
        # DVE never idles across the level transition
        l1A, l1B, l1sched = make_level(
            1, N1G, N0G, 16, 64, h2nats,
            {g: posts[(g, 1, "p")] for g in GS},
            {g: posts[(g, 1, "q")] for g in GS},
            qsq1, P["xs0T"], 3,
            [(W1aT, b1a, True, 64, bf16),
             (W1bT, b1b, True, 64, bf16),
             (W1cT, b1c, False, 3, f32)], outTs)
        l2A, l2B, l2sched = make_level(
            2, N2G, N1G, 8, 128, h3nats,
            {g: posts[(g, 2, "p")] for g in GS},
            {g: posts[(g, 2, "q")] for g in GS},
            {g: qsq32[(g, 2)] for g in GS}, P["xs1T"], 64,
            [(W2aT, b2a, True, 64, bf16),
             (W2bT, b2b, False, 64, bf16)], h2Ts)
        l1pre = [(0, 0), (1, 0), (0, 1), (1, 1), (0, 2), (1, 2)]
        pre_at = {1: 0, 2: 1, 3: 2, 4: 3, 5: 4, 6: 5}
        for i, (g, pi) in enumerate(l2sched):
            l2A(g, pi)
            l2B(g, pi)
            if i in pre_at:
                l1A(*l1pre[pre_at[i]])
        for g in GS:
            h2nat = []
            for j in range(8):
                hn = gpool.tile([128, 64], bf16, tag=f"h2n{j}",
                                name=f"h2n{j}_g{g}")
                nc.sync.dma_start_transpose(
                    hn[:, :], h2Ts[g][:, j * 128 : (j + 1) * 128])
                h2nat.append((hn, 128))
            h2nats[g] = h2nat

        # B lags A so the kernel tail is agg/MLP-only (no trailing dense
        # DVE chain after the last weight production)
        lagq = []
        for g, pi in l1sched:
            if (g, pi) not in l1pre:
                l1A(g, pi)
            lagq.append((g, pi))
            if len(lagq) > 1:
                l1B(*lagq.pop(0))
        for e in lagq:
            l1B(*e)
        for g in GS:
            for qi in range(8):
                c0, c1 = qi * 512, (qi + 1) * 512
                eng = nc.sync if (g + qi) % 2 == 0 else nc.scalar
                eng.dma_start(P["out"].ap()[g, :, c0:c1],
                              outTs[g][:, c0:c1])

    return nc, P


_NC = None


def _get_nc():
    global _NC
    if _NC is None:
        nc = build_module()[0]
        nc.finalize()  # Bacc lowering: EVSEM wait legalization + reg alloc
        _NC = nc
    return _NC


def _hilo(a):
    """fp16 hi/lo split: a ~= hi + lo with products of halves exact in f32."""
    hi = a.astype(np.float16)
    lo = (a - hi.astype(np.float32)).astype(np.float16)
    return hi, lo


def _pack_side(pos, ng, is_q):
    """[2*ng, 3] f32 -> [2, 11, ng] fp16 matmul operand rows.

    p-side rows: [pxh pxl pxh  pyh pyl pyh  pzh pzl pzh  0 0]
      (rows 9,10 = -|p|^2/2 hi/lo, written on device)
    q-side rows: [qxh qxh qxl  qyh qyh qyl  qzh qzh qzl  1 1]
      (the -|q|^2/2 term is added per-partition at PSUM->SBUF staging)
    """
    c = pos.reshape(2, ng, 3).transpose(0, 2, 1)  # [2, 3, ng]
    hi, lo = _hilo(c)
    t = np.zeros((2, KROWS, ng), np.float16)
    for d in range(3):
        if is_q:
            t[:, 3 * d + 0] = hi[:, d]
            t[:, 3 * d + 1] = hi[:, d]
            t[:, 3 * d + 2] = lo[:, d]
        else:
            t[:, 3 * d + 0] = hi[:, d]
            t[:, 3 * d + 1] = lo[:, d]
            t[:, 3 * d + 2] = hi[:, d]
    if is_q:
        t[:, 9] = 1.0
        t[:, 10] = 1.0
    return np.ascontiguousarray(t)


def shard_inputs(inputs):
    f = lambda name: np.ascontiguousarray(np.asarray(inputs[name], np.float32))
    pos_arrs = {
        "pos": (f("pos"), N3G), "ps2": (f("pos_skip2"), N2G),
        "ps1": (f("pos_skip1"), N1G), "ps0": (f("pos_skip0"), N0G),
    }
    skips = {
        "xs2T": (f("x_skip2"), N2G), "xs1T": (f("x_skip1"), N1G),
        "xs0T": (f("x_skip0"), N0G),
    }
    weights = {k: f(k) for k in ["W3a", "b3a", "W3b", "b3b", "W2a", "b2a",
                                 "W2b", "b2b", "W1a", "b1a", "W1b", "b1b",
                                 "W1c", "b1c"]}
    # (param, source, is_q)
    sides = [("pT3", "pos", False), ("qT3", "ps2", True),
             ("pT2", "ps2", False), ("qT2", "ps1", True),
             ("pT1", "ps1", False), ("qT1", "ps0", True)]
    x_full = f("x")
    in_maps = []
    for c in range(N_CORES):
        m = dict(weights)
        sub_pos = {}
        for nm, (arr, ng) in pos_arrs.items():
            sub = np.ascontiguousarray(arr[2 * c * ng : (2 * c + 2) * ng])
            m[nm] = sub
            sub_pos[nm] = (sub, ng)
        for pnm, src, is_q in sides:
            sub, ng = sub_pos[src]
            m[pnm] = _pack_side(sub, ng, is_q)
        for nm, (arr, ng) in skips.items():
            sub = arr[2 * c * ng : (2 * c + 2) * ng]
            d = sub.shape[1]
            t = sub.reshape(2, ng, d).transpose(0, 2, 1)
            m[nm] = np.ascontiguousarray(t.astype(ml_dtypes.bfloat16))
        m["x"] = np.ascontiguousarray(
            x_full[2 * c * N3G : (2 * c + 2) * N3G].astype(ml_dtypes.bfloat16))
        in_maps.append(m)
    return in_maps


def kernel(**inputs):
    nc = _get_nc()
    in_maps = shard_inputs(inputs)
    from concourse.bass_utils import run_bass_kernel_spmd

    res = run_bass_kernel_spmd(nc, in_maps, list(range(N_CORES)))
    # device writes [g, 3, n]; restore the [n_total, 3] layout
    return np.concatenate(
        [np.asarray(r["out"], np.float32).transpose(0, 2, 1).reshape(-1, 3)
         for r in res.results], axis=0)


if __name__ == "__main__":
    nc, _ = build_module()
    print("build ok")


# revision 47
# speedup vs baseline: 1.0100x; 1.0018x over previous
"""Trainium2 Bass kernel for nn_DecoderPp (PointNet++-style 3-level KNN decoder).

Data-parallel over 16 graphs: core c owns graphs 2c, 2c+1; tiles of the
two graphs interleave within each level. Per 256-target tile PAIR:
- PE computes m = q.p - |p|^2/2 via a K=11 fp16 matmul: coordinates are
  hi/lo split into two fp16 halves (q.p = qh.ph + qh.pl + ql.ph, products
  exact in the f32 accumulator; dropped ql.pl ~2^-22), so the matmul runs
  at 1 cyc/col like bf16 with ~3e-5 absolute error -- 50x tighter than
  f32r (whose rounding flips neighbor selections) at 1/4 the PE cost of
  f32. Coordinate rows are host-relayout params; the p-side -|p|^2/2 fp16
  hi/lo rows are built on device (batched DVE square+reduce + PE transpose
  + row DMAs). The per-target -|q|^2/2 stays NATURAL [128, a] f32 and
  rides the PSUM->SBUF ACT staging copy as its per-partition bias -- no
  transpose, no wide single-partition-row DMA (those cost ~0.39 ns/byte
  of per-partition payload, 3.2us for a [1,4096] f32 row).
- Selection: k<=8 is one DVE max8; k=16 takes per-block max8s (5 blocks)
  whose top-8 union covers the true top-16 except ~1e-3/row (a miss only
  lowers tau -> an extra small-weight neighbor), then an exact
  top-16-of-union merge (max8 + match_replace + max8 on the 40-col tile).
  tau flows bit-exactly (max8 copies f32) so no epsilon bump is needed.
- Weights: ONE fused custom-DVE pass (registered at import into
  concourse.dve_ops) MASKED_RECIP_SUM_DECPP: Wraw = (s >= tau) ?
  recip1nr(s) : 0 with accum sw, where recip1nr = BITWISE_NOT bit-trick
  seed + one Newton step (~0.4% error, same class as the bf16 weight
  cast). Replaces the dense InstReciprocal + masked stt pair; HW-checked
  (Pool has no generic elementwise ISA and DVE has no divide ALU op, so
  this is the only single-pass form). Pool's normalize_recip ucode then
  divides by sw and casts to bf16 off the DVE/ACT critical path.
- Per-128 xbar DMA transposes feed bf16 aggregation matmuls y^T = xe^T
  W^T; consecutive same-graph tiles PAIR through agg+MLP so every ACT op
  there runs [*, 256], halving its fixed SBUF-access overhead. MLP is
  feature-major on PE, tanh/bias fused into ACT. Skips are host-relayout
  bf16 [Ck, n] params sliced per pair.
- Levels are split into pair_A (weight production, positions only) and
  pair_B (agg+MLP, needs previous level's features): level 1's first four
  pair_As pre-warm inside the level-2 stream and its pair_Bs lag two
  pairs so the kernel tail is agg/MLP-only. PSUM: s 2x2 banks + y 2 + mlp
  2 rings.
Cost-model time: 220752 ns (session start 309335, original 500480);
hw rel err ~6.4e-3.
Built on Bacc (finalize() legalizes multi-semaphore waits via EVSEM).
"""
import sys
from contextlib import ExitStack

if "/opt/trn_rl_repo" not in sys.path:
    sys.path.insert(0, "/opt/trn_rl_repo")

import ml_dtypes
import numpy as np

import concourse.bass as bass
import concourse.mybir as mybir
from concourse.bacc import Bacc
from concourse.tile import TileContext
from concourse.masks import make_identity

dt = mybir.dt
AF = mybir.ActivationFunctionType
ALU = mybir.AluOpType
AX = mybir.AxisListType

N_CORES = 8
GRAPHS_PER_CORE = 2
N3G, N2G, N1G, N0G = 64, 256, 1024, 4096  # per-graph sizes per level

NEG_BIG = -1.0e30
TAU_BUMP = 1.0 + 1.0e-6  # tau' = tau*(1+1e-6): k-th (negative) value stays selected
KROWS = 11  # 9 coord hi/lo product rows + 2 p-side sq (or ones) rows

f32 = dt.float32
f16 = dt.float16
bf16 = dt.bfloat16

USE_CUSTOM_DVE = True

# ---- fused masked-reciprocal-sum custom DVE op -------------------------
# out = (s >= tau) ? recip1nr(s) : 0 ; accum_out = sum(out). Replaces the
# dense InstReciprocal + masked scalar_tensor_tensor pair with ONE DVE
# pass. recip1nr is the BITWISE_NOT bit-trick seed (s * bitcast(~s) lands
# in [-4.5, -4] for either sign) + one Newton step: ~0.4% relative error,
# the same class as the bf16 weight cast the baseline already used. The
# building blocks (select-by-compare, BITWISE_NOT seed, add-accumulate)
# all appear in HW-validated registry ops.
_MRS_C0 = -0.23549792  # Chebyshev seed scale over [-4.5, -4]
_MRS_C1 = 2.0017324    # Newton c1

from concourse import dve_ops as _dve_ops
from concourse import dve_spec as _dve_spec


def _make_masked_recip_sum():
    name = "MASKED_RECIP_SUM_DECPP"
    if name in _dve_ops.CUSTOM_DVE_SPECS:
        return next(o for o in _dve_ops.OPS if o.name == name)
    Src0, C0, C1, C2 = (_dve_spec.Src0, _dve_spec.C0, _dve_spec.C1,
                        _dve_spec.C2)
    from operator import add as _add

    _not = _dve_spec.Bin(_dve_spec.AluOp.BITWISE_NOT, Src0, Src0)
    _y0 = _not * C1
    _y1 = _y0 * (C2 - Src0 * _y0)

    def _ref(in0, in1, s0, s1, imm2):
        x = np.ascontiguousarray(in0, dtype=np.float32)
        not_x = (~x.view(np.int32)).view(np.float32)
        y0 = not_x * s1
        y1 = (y0 * (imm2 - x * y0)).astype(np.float32)
        b = np.where(x >= s0, y1, np.float32(0.0)).astype(np.float32)
        return b, b.reshape(b.shape[0], -1).sum(axis=-1, keepdims=True)

    spec = _dve_spec.Spec(
        body=_dve_spec.select(Src0 >= C0, _y1, _dve_spec.Zero),
        accum=_add,
        accum_init=_dve_spec.Zero,
        reference=_ref,
    )
    ver = "v3"
    lowered = _dve_ops.DveOpSpec(
        name=name,
        opcode=_dve_ops._CUSTOM_DVE_ROW_BASE + len(_dve_ops.OPS),
        uops=_dve_spec.lower(spec, ver=ver),
        rd1_en=_dve_spec._has_src1(spec),
    )
    op = _dve_ops.DveOp(name, spec, False, {ver: lowered.sha(ver)})
    _dve_ops.OPS.append(op)
    _dve_ops.CUSTOM_DVE_SPECS[name] = spec
    _dve_ops._SUB_OPCODE_FOR_NAME[name] = (
        _dve_ops._CUSTOM_DVE_ROW_BASE + len(_dve_ops.OPS) - 1)
    return op


MASKED_RECIP_SUM = _make_masked_recip_sum()


def _ceil_div(a, b):
    return (a + b - 1) // b


def build_module():
    nc = Bacc()

    P = {}

    def param(name, shape, out=False, dtype=f32):
        P[name] = nc.declare_dram_parameter(name, list(shape), dtype,
                                            isOutput=out)

    param("x", (GRAPHS_PER_CORE * N3G, 256), dtype=bf16)
    param("pos", (GRAPHS_PER_CORE * N3G, 3))
    param("ps2", (GRAPHS_PER_CORE * N2G, 3))
    param("ps1", (GRAPHS_PER_CORE * N1G, 3))
    param("ps0", (GRAPHS_PER_CORE * N0G, 3))
    # host-relayout fp16 matmul operand tiles [g, 13, n]: coordinate hi/lo
    # product rows + ones rows; sq rows (9,10 on p-side / 11,12 on q-side)
    # are zero-filled by the host and written on device
    param("pT3", (GRAPHS_PER_CORE, KROWS, N3G), dtype=f16)
    param("qT3", (GRAPHS_PER_CORE, KROWS, N2G), dtype=f16)
    param("pT2", (GRAPHS_PER_CORE, KROWS, N2G), dtype=f16)
    param("qT2", (GRAPHS_PER_CORE, KROWS, N1G), dtype=f16)
    param("pT1", (GRAPHS_PER_CORE, KROWS, N1G), dtype=f16)
    param("qT1", (GRAPHS_PER_CORE, KROWS, N0G), dtype=f16)
    param("xs2T", (GRAPHS_PER_CORE, 128, N2G), dtype=bf16)
    param("xs1T", (GRAPHS_PER_CORE, 64, N1G), dtype=bf16)
    param("xs0T", (GRAPHS_PER_CORE, 3, N0G), dtype=bf16)
    for nm, shp in [
        ("W3a", (128, 384)), ("b3a", (128,)),
        ("W3b", (128, 128)), ("b3b", (128,)),
        ("W2a", (64, 192)), ("b2a", (64,)),
        ("W2b", (64, 64)), ("b2b", (64,)),
        ("W1a", (64, 67)), ("b1a", (64,)),
        ("W1b", (64, 64)), ("b1b", (64,)),
        ("W1c", (3, 64)), ("b1c", (3,)),
    ]:
        param(nm, shp)
    param("out", (GRAPHS_PER_CORE, 3, N0G), out=True)

    with TileContext(nc) as tc, ExitStack() as ctx:
        consts = ctx.enter_context(tc.tile_pool(name="consts", bufs=1))
        wpool = ctx.enter_context(tc.tile_pool(name="weights", bufs=1))
        gpool = ctx.enter_context(tc.tile_pool(name="graph", bufs=2))
        tpool = ctx.enter_context(tc.tile_pool(name="tiles", bufs=7))
        npool = ctx.enter_context(tc.tile_pool(name="narrow", bufs=12))
        # PSUM budget (8 banks x 2KB): s 2x2 banks + y 2x1 + mlp 2x1 = 8.
        # 2-deep mlp/y rings keep consecutive tiles' MLP layers from
        # serializing through a single PE<->ACT buffer.
        pspool = ctx.enter_context(tc.tile_pool(name="ps_s", bufs=2, space="PSUM"))
        psy = ctx.enter_context(tc.tile_pool(name="ps_y", bufs=2, space="PSUM"))
        psm = ctx.enter_context(tc.tile_pool(name="ps_mlp", bufs=2, space="PSUM"))

        ident0 = consts.tile([128, 128], f32)
        make_identity(nc, ident0)
        # ACT-written copy: PE transposes read this so their input waits
        # collapse onto the Activation semaphore (walrus LDW 1-wait limit)
        ident = consts.tile([128, 128], f32)
        nc.scalar.activation(ident[:, :], ident0[:, :], AF.Copy)

        # ---- weight prep: transposed chunks + f32 bias columns ----
        def prep_linear(wname, bname, O, I, splits, wdtype=bf16, q="gpsimd"):
            eng = getattr(nc, q)
            w_sb = wpool.tile([O, I], f32, tag=f"{wname}_raw")
            eng.dma_start(w_sb[:, :], P[wname].ap())
            chunks = []
            c0 = 0
            for j, cw in enumerate(splits):
                c1 = c0 + cw
                ps_t = psm.tile([128, 128], f32, tag="mlp")
                nc.tensor.transpose(ps_t[:cw, :O], w_sb[:, c0:c1],
                                    ident[:O, :O])
                wt = wpool.tile([cw, O], wdtype, tag=f"{wname}T{j}")
                nc.scalar.activation(wt[:, :], ps_t[:cw, :O], AF.Copy)
                chunks.append((wt, cw))
                c0 = c1
            bcol = wpool.tile([O, 1], f32, tag=f"{bname}col")
            eng.dma_start(bcol[:, :], P[bname].ap())
            return chunks, bcol

        def make_posT_load(dramT, g, n, tag, eng=None):
            """One [13,n] fp16 DMA loads coord hi/lo + ones rows (host
            layout); the sq chain later fills the side's sq rows. ACT DMA
            channel keeps SP free for per-tile transposes."""
            pt = gpool.tile([KROWS, n], f16, tag=tag, name=f"pt_{tag}_g{g}")
            (eng if eng is not None else nc.scalar).dma_start(
                pt[:, :], dramT.ap()[g, :, :])
            return pt

        P_HI, P_LO = 9, 10  # p-side sq row indices in the operand tiles

        def stage_sq_graph(uid, g, specs, dma_eng=None):
            """Batched -|.|^2/2 for every pos tensor of graph g.
            specs: list of (pt_or_None, dram, n, kind). One wide DVE
            square+reduce+scale for all; p-side ("p") entries get fp16
            hi/lo rows DMA'd into their operand tile (rows 9,10); q-side
            ("q") entries stay NATURAL: their per-target -|q|^2/2 columns
            ride the PSUM->SBUF staging copy as a free ACT bias.
            Returns {spec_index: (s2h_tile, col0)} for q-sides."""
            groups = []  # (pt, row0, a, n, kind)
            row0 = 0
            for pt, dram, n, kind in specs:
                a = max(1, n // 128)
                groups.append((pt, row0, a, n, kind))
                row0 += a
            atot = row0
            nball = gpool.tile([128, atot * 3], f32, tag=f"nball{uid % 2}",
                               name=f"nball_u{uid}")
            nc.vector.memset(nball[:, :], 0.0)
            for (pt, r0, a, n, kind), spec in zip(groups, specs):
                dram = spec[1]
                base = g * n
                if n >= 128:
                    src_ap = dram.ap()[base : base + n, :].rearrange(
                        "(a p) d -> p a d", p=128)
                    nc.sync.dma_start(nball[:, 3 * r0 : 3 * (r0 + a)], src_ap)
                else:
                    nc.sync.dma_start(nball[:n, 3 * r0 : 3 * r0 + 3],
                                      dram.ap()[base : base + n, :])
            sq = gpool.tile([128, atot * 3], f32, tag=f"sqall{uid % 2}",
                            name=f"sqall_u{uid}")
            nc.vector.tensor_tensor(sq[:, :], nball[:, :], nball[:, :],
                                    op=ALU.mult)
            s2 = gpool.tile([128, atot], f32, tag=f"s2all{uid % 2}",
                            name=f"s2all_u{uid}")
            nc.vector.tensor_reduce(
                s2[:, :], sq[:, :].rearrange("p (a d) -> p a d", d=3),
                axis=AX.X, op=ALU.add)
            s2h = gpool.tile([128, atot], f32, tag=f"s2hall{uid % 2}",
                             name=f"s2hall_u{uid}")
            nc.vector.tensor_scalar(s2h[:, :], s2[:, :], -0.5, None,
                                    op0=ALU.mult)
            p_atot = sum(a for _, _, a, _, kind in groups if kind == "p")
            qcols = {}
            if p_atot:
                t_ps = psm.tile([128, 128], f32, tag="mlp")
                nc.tensor.transpose(t_ps[:atot, :], s2h[:, :], ident[:, :])
                # fp16 hi/lo split of the p-side sq rows (error ~2^-22|p|^2)
                s2hiT = gpool.tile([64, 128], f16, tag=f"s2hiT{uid % 2}",
                                   name=f"s2hiT_u{uid}")
                nc.scalar.activation(s2hiT[:atot, :], t_ps[:atot, :], AF.Copy)
                s2loT = gpool.tile([64, 128], f16, tag=f"s2loT{uid % 2}",
                                   name=f"s2loT_u{uid}")
                with nc.allow_low_precision("fp16 lo residual of hi/lo "
                                            "split"):
                    nc.vector.tensor_tensor(s2loT[:atot, :], t_ps[:atot, :],
                                            s2hiT[:atot, :], op=ALU.subtract)
            eng = dma_eng if dma_eng is not None else nc.gpsimd
            for i, (pt, r0, a, n, kind) in enumerate(groups):
                if kind == "q":
                    qcols[i] = (s2h, r0)
                    continue
                if n >= 128:
                    eng.dma_start(pt[P_HI : P_HI + 1, :],
                                  s2hiT[r0 : r0 + a, :])
                    eng.dma_start(pt[P_LO : P_LO + 1, :],
                                  s2loT[r0 : r0 + a, :])
                else:
                    eng.dma_start(pt[P_HI : P_HI + 1, :],
                                  s2hiT[r0 : r0 + 1, :n])
                    eng.dma_start(pt[P_LO : P_LO + 1, :],
                                  s2loT[r0 : r0 + 1, :n])
            return qcols

        # ---------------- one interpolation+MLP level ----------------
        def make_level(lvl, ns, nt, k, Cs, xe_chunks, pTs, qTs, qsqs,
                       skipT_dram, Ck, mlp, out_tiles):
            """Returns (pair_A, pair_B, sched): pair_A(g, pi) produces the
            normalized weight transposes for one 256-target pair (needs only
            positions); pair_B(g, pi) aggregates + runs the MLP (needs the
            previous level's features). Splitting lets a later level's A
            phase pre-warm during the previous level's drain. Pairs of
            consecutive same-graph tiles share the agg/MLP stage so every
            ACT op there runs at [*, 256], halving its fixed SBUF-access
            overhead per tile. mlp: list of (chunks, bcol, tanh?, O, odt)."""

            ns_pad = max(128, ns)
            n_sch = _ceil_div(ns, 128)
            nfc = _ceil_div(Cs, 128)
            gs = list(range(GRAPHS_PER_CORE))
            ntile = nt // 128
            npair = ntile // 2
            sched = [(gs[i % len(gs)], i // len(gs))
                     for i in range(len(gs) * npair)]
            WT_store = {}

            def pair_A(g, pi):
                pT, qT = pTs[g], qTs[g]
                qsq, qc0 = qsqs[g]
                WTs = {}
                for half in range(2):
                    ti = pi * 2 + half
                    t0 = ti * 128
                    # sb = -d^2/2 : [128, ns] PSUM (K=13 fp16 matmul)
                    s_ps = pspool.tile([128, 1024], f32, tag="s")
                    qlhs = qT[:, t0 : t0 + 128]
                    for h0 in range(0, ns, 512):
                        h1 = min(ns, h0 + 512)
                        nc.tensor.matmul(s_ps[:, h0:h1], qlhs,
                                         pT[:, h0:h1],
                                         start=True, stop=True)

                    # ACT stages sb to SBUF: DVE pays the cheap SBUF access
                    # latency and Pool (no PSUM port) can read it. The
                    # per-target -|q|^2/2 term rides along as the (free)
                    # per-partition bias column -- it never touches the
                    # matmul or a row DMA.
                    s_sb = tpool.tile([128, ns_pad], f32, tag="s_sb")
                    nc.scalar.activation(s_sb[:, :ns], s_ps[:, :ns],
                                         AF.Identity,
                                         bias=qsq[:, qc0 + ti : qc0 + ti + 1])

                    # --- selection: tau = k-th largest sb per row.
                    # Per-block top-8s (5 blocks) cover the true top-16
                    # except ~1e-3/row (misses only ADD a small extra
                    # neighbor: tau from the union is <= the true k-th, so
                    # the dense compare still keeps all true neighbors);
                    # exact top-16 of the union. The k-th value flows
                    # bit-exactly (max8 copies f32) so the compare needs no
                    # epsilon bump. ---
                    if k == 16:
                        nb = 5
                        bounds = [round(i * ns / nb) for i in range(nb + 1)]
                        v8s = tpool.tile([128, 8 * nb], f32, tag="v8s")
                        for j in range(nb):
                            nc.vector.max(v8s[:, 8 * j : 8 * j + 8],
                                          s_sb[:, bounds[j] : bounds[j + 1]])
                        m16 = npool.tile([128, 16], f32, tag="m16")
                        nc.vector.max(m16[:, 0:8], v8s[:, :])
                        zapc = tpool.tile([128, 8 * nb], f32, tag="zapc")
                        nc.vector.match_replace(zapc[:, :], m16[:, 0:8],
                                                v8s[:, :], NEG_BIG)
                        nc.vector.max(m16[:, 8:16], zapc[:, :])
                        taur = m16[:, 15:16]
                    else:
                        v8 = npool.tile([128, 8], f32, tag="v8")
                        nc.vector.max(v8[:, :], s_sb[:, :ns])
                        taur = v8[:, k - 1 : k]

                    # --- weights: Wraw = (sb >= taur) * (1/sb), accum sw ---
                    Wraw = tpool.tile([128, ns_pad], f32, tag="Wraw")
                    sw = npool.tile([128, 1], f32, tag="sw")
                    if USE_CUSTOM_DVE:
                        # one fused DVE pass: bit-trick reciprocal seed + one
                        # NR step (~0.4% wrec error, same class as the bf16
                        # weight cast) + threshold select + sum accumulate
                        nc.vector._custom_dve(
                            MASKED_RECIP_SUM, out=Wraw[:, :ns],
                            in0=s_sb[:, :ns], s0=taur,
                            s1=_MRS_C0, imm2=_MRS_C1,
                            accum_out=sw[:, :])
                    else:
                        # two proven DVE passes (dense reciprocal + masked
                        # multiply-accumulate)
                        wrec = tpool.tile([128, ns_pad], bf16, tag="wrec")
                        with nc.allow_low_precision("inverse-distance "
                                                    "weights; bf16 ok"):
                            nc.vector.reciprocal(wrec[:, :ns], s_sb[:, :ns])
                        nc.vector.scalar_tensor_tensor(
                            Wraw[:, :ns], s_sb[:, :ns], taur,
                            wrec[:, :ns], op0=ALU.is_ge, op1=ALU.mult,
                            accum_out=sw[:, :])
                    # Pool ucode normalizes and casts: W = Wraw / sw -> bf16
                    W = tpool.tile([128, ns_pad], bf16, tag="W")
                    if ns < ns_pad:
                        nc.vector.memset(W[:, ns:], 0.0)
                    nc.gpsimd.normalize_recip(W[:, :ns], Wraw[:, :ns],
                                              sw[:, :])

                    # --- transpose W chunks for aggregation ---
                    WT = []
                    for j in range(ns_pad // 128):
                        wt = tpool.tile([128, 128], bf16, tag=f"WT{half}_{j}")
                        nc.sync.dma_start_transpose(
                            wt[:, :], W[:, j * 128 : (j + 1) * 128])
                        WT.append(wt)
                    WTs[half] = WT
                WT_store[(g, pi)] = WTs

            def pair_B(g, pi):
                out_tile = out_tiles[g]
                p0 = pi * 256
                WTs = WT_store.pop((g, pi))
                # --- aggregate y^T = xe^T @ W^T (both halves, [*, 256]) ---
                y_ps = []
                for fc in range(nfc):
                    f0, f1 = fc * 128, min(Cs, (fc + 1) * 128)
                    yp = psy.tile([128, 256], f32, tag="y")
                    for half in range(2):
                        c0 = half * 128
                        for j in range(n_sch):
                            kr = min(128, ns - j * 128)
                            nc.tensor.matmul(yp[: f1 - f0, c0 : c0 + 128],
                                             xe_chunks[g][j][0][:kr, f0:f1],
                                             WTs[half][j][:kr, :],
                                             start=(j == 0),
                                             stop=(j == n_sch - 1))
                    y_ps.append((yp, f1 - f0))

                # --- MLP input chunks: y^T (bf16) + skip^T ---
                in_chunks = []
                for fc, (yp, fw) in enumerate(y_ps):
                    hc = tpool.tile([128, 256], bf16, tag=f"hc{fc}")
                    nc.scalar.activation(hc[:fw, :], yp[:fw, :], AF.Copy)
                    in_chunks.append((hc, fw))
                skc = tpool.tile([Ck, 256], bf16, tag="skc")
                nc.sync.dma_start(skc[:, :],
                                  skipT_dram.ap()[g, :, p0 : p0 + 256])
                in_chunks.append((skc, Ck))

                # --- MLP (feature-major, [*, 256]) ---
                cur = in_chunks
                for li, (chunks, bcol, tanh, O, odt) in enumerate(mlp):
                    mp = psm.tile([128, 256], f32, tag="mlp")
                    nkc = len(cur)
                    for j, (ct, kr) in enumerate(cur):
                        wt, cw = chunks[j]
                        assert cw == kr, f"l{lvl} mlp{li} c{j}: {cw} != {kr}"
                        nc.tensor.matmul(mp[:O, :], wt[:, :O], ct[:kr, :],
                                         start=(j == 0), stop=(j == nkc - 1))
                    if li == len(mlp) - 1:
                        nc.scalar.activation(out_tile[:O, p0 : p0 + 256],
                                             mp[:O, :], AF.Identity,
                                             bias=bcol[:, :])
                    else:
                        ho = tpool.tile([128, 256], odt, tag=f"ho{li}")
                        nc.scalar.activation(ho[:O, :], mp[:O, :],
                                             AF.Tanh if tanh else AF.Identity,
                                             bias=bcol[:, :])
                        cur = [(ho, O)]

            return pair_A, pair_B, sched

        def prop_level(*args, pre=()):
            """Emit one level; `pre` pairs already had pair_A emitted."""
            pair_A, pair_B, sched = make_level(*args)
            for g, pi in sched:
                if (g, pi) not in pre:
                    pair_A(g, pi)
                pair_B(g, pi)

        # ---------------- per-graph pipeline ----------------
        # All pos tensors (including level 1) are loaded up front, chunked
        # and spread across the ACT/Pool DMA queues so no queue holds a
        # multi-us transfer in front of compute-critical work.
        posts = {}
        xe3s = {}
        for g in range(GRAPHS_PER_CORE):
            xe3 = gpool.tile([64, 256], bf16, tag="xe3", name=f"xe3_g{g}")
            nc.sync.dma_start(xe3[:, :], P["x"].ap()[g * 64 : (g + 1) * 64, :])
            xe3s[g] = xe3
        # levels 3+2 pos tensors (small direct [11, n] loads)
        for lvl, (pd, pn, qd, qn) in {
            3: ("pT3", N3G, "qT3", N2G),
            2: ("pT2", N2G, "qT2", N1G),
        }.items():
            for g in range(GRAPHS_PER_CORE):
                posts[(g, lvl, "p")] = make_posT_load(P[pd], g, pn,
                                                      f"pT{lvl}")
                posts[(g, lvl, "q")] = make_posT_load(P[qd], g, qn,
                                                      f"qT{lvl}")
        # level-1 pos tensors: pT1 direct; qT1 [11, 4096] split into 4
        # column chunks alternating between the Pool and ACT queues
        for g in range(GRAPHS_PER_CORE):
            posts[(g, 1, "p")] = make_posT_load(P["pT1"], g, N1G, "pT1",
                                                eng=nc.gpsimd)
            qt = gpool.tile([KROWS, N0G], f16, tag="qT1", name=f"pt_qT1_g{g}")
            for ci in range(4):
                c0, c1 = ci * 1024, (ci + 1) * 1024
                eng = nc.gpsimd if (ci + g) % 2 == 0 else nc.scalar
                eng.dma_start(qt[:, c0:c1], P["qT1"].ap()[g, :, c0:c1])
            posts[(g, 1, "q")] = qt

        # batched sq chains; q-sides stay natural for the staging bias
        qsq32 = {}
        qsq1 = {}
        for g in range(GRAPHS_PER_CORE):
            specs = [
                (posts[(g, 3, "p")], P["pos"], N3G, "p"),
                (None, P["ps2"], N2G, "q"),
                (posts[(g, 2, "p")], P["ps2"], N2G, "p"),
                (None, P["ps1"], N1G, "q"),
            ]
            qc = stage_sq_graph(g, g, specs)
            qsq32[(g, 3)] = qc[1]
            qsq32[(g, 2)] = qc[3]
        for g in range(GRAPHS_PER_CORE):
            specs = [
                (posts[(g, 1, "p")], P["ps1"], N1G, "p"),
                (None, P["ps0"], N0G, "q"),
            ]
            qc = stage_sq_graph(2 + g, g, specs)
            qsq1[g] = qc[1]

        W3aT, b3a = prep_linear("W3a", "b3a", 128, 384, [128, 128, 128],
                                q="sync")
        W3bT, b3b = prep_linear("W3b", "b3b", 128, 128, [128], q="sync")
        W2aT, b2a = prep_linear("W2a", "b2a", 64, 192, [128, 64])
        W2bT, b2b = prep_linear("W2b", "b2b", 64, 64, [64])
        W1aT, b1a = prep_linear("W1a", "b1a", 64, 67, [64, 3])
        W1bT, b1b = prep_linear("W1b", "b1b", 64, 64, [64])
        W1cT, b1c = prep_linear("W1c", "b1c", 3, 64, [64])

        h3Ts, h3nats, h2Ts, h2nats, outTs = {}, {}, {}, {}, {}
        GS = list(range(GRAPHS_PER_CORE))
        for g in GS:
            h3Ts[g] = gpool.tile([128, 256], bf16, tag="h3T", name=f"h3T_g{g}")
            h2Ts[g] = gpool.tile([64, 1024], bf16, tag="h2T", name=f"h2T_g{g}")
            outTs[g] = gpool.tile([3, 4096], f32, tag="outT", name=f"outT_g{g}")
        # h3nats/h2nats are filled AFTER their producing level; the later
        # level's pair_B only dereferences them at its own emission time.
        prop_level(3, N3G, N2G, 4, 256, {g: [(xe3s[g], 64)] for g in GS},
                   {g: posts[(g, 3, "p")] for g in GS},
                   {g: posts[(g, 3, "q")] for g in GS},
                   {g: qsq32[(g, 3)] for g in GS}, P["xs2T"], 128,
                   [(W3aT, b3a, True, 128, bf16),
                    (W3bT, b3b, False, 128, bf16)], h3Ts)
        for g in GS:
            h3nat = []
            for j in range(2):
                hn = gpool.tile([128, 128], bf16, tag=f"h3n{j}",
                                name=f"h3n{j}_g{g}")
                nc.sync.dma_start_transpose(
                    hn[:, :], h3Ts[g][:, j * 128 : (j + 1) * 128])
                h3nat.append((hn, 128))
            h3nats[g] = h3nat


        # level 1 is W-heavy on DVE: pre-warm its first pairs' weight
        # production (phase A, position-only) inside the level-2 stream so
        # DVE never idles across the level transition
        l1A, l1B, l1sched = make_level(
            1, N1G, N0G, 16, 64, h2nats,
            {g: posts[(g, 1, "p")] for g in GS},
            {g: posts[(g, 1, "q")] for g in GS},
            qsq1, P["xs0T"], 3,
            [(W1aT, b1a, True, 64, bf16),
             (W1bT, b1b, True, 64, bf16),
             (W1cT, b1c, False, 3, f32)], outTs)
        l2A, l2B, l2sched = make_level(
            2, N2G, N1G, 8, 128, h3nats,
            {g: posts[(g, 2, "p")] for g in GS},
            {g: posts[(g, 2, "q")] for g in GS},
            {g: qsq32[(g, 2)] for g in GS}, P["xs1T"], 64,
            [(W2aT, b2a, True, 64, bf16),
             (W2bT, b2b, False, 64, bf16)], h2Ts)
        l1pre = [(0, 0), (1, 0), (0, 1), (1, 1)]
        pre_at = {1: 0, 3: 1, 5: 2, 7: 3}
        for i, (g, pi) in enumerate(l2sched):
            l2A(g, pi)
            l2B(g, pi)
            if i in pre_at:
                l1A(*l1pre[pre_at[i]])
        for g in GS:
            h2nat = []
            for j in range(8):
                hn = gpool.tile([128, 64], bf16, tag=f"h2n{j}",
                                name=f"h2n{j}_g{g}")
                nc.sync.dma_start_transpose(
                    hn[:, :], h2Ts[g][:, j * 128 : (j + 1) * 128])
                h2nat.append((hn, 128))
            h2nats[g] = h2nat

        # B lags A so the kernel tail is agg/MLP-only (no trailing dense
        # DVE chain after the last weight production)
        lagq = []
        for g, pi in l1sched:
            if (g, pi) not in l1pre:
                l1A(g, pi)
            lagq.append((g, pi))
            if len(lagq) > 2:
                l1B(*lagq.pop(0))
        for e in lagq:
            l1B(*e)
        for g in GS:
            for qi in range(8):
                c0, c1 = qi * 512, (qi + 1) * 512
                eng = nc.sync if (g + qi) % 2 == 0 else nc.scalar
                eng.dma_start(P["out"].ap()[g, :, c0:c1],
                              outTs[g][:, c0:c1])

    return nc, P


_NC = None


def _get_nc():
    global _NC
    if _NC is None:
        nc = build_module()[0]
        nc.finalize()  # Bacc lowering: EVSEM wait legalization + reg alloc
        _NC = nc
    return _NC


def _hilo(a):
    """fp16 hi/lo split: a ~= hi + lo with products of halves exact in f32."""
    hi = a.astype(np.float16)
    lo = (a - hi.astype(np.float32)).astype(np.float16)
    return hi, lo


def _pack_side(pos, ng, is_q):
    """[2*ng, 3] f32 -> [2, 11, ng] fp16 matmul operand rows.

    p-side rows: [pxh pxl pxh  pyh pyl pyh  pzh pzl pzh  0 0]
      (rows 9,10 = -|p|^2/2 hi/lo, written on device)
    q-side rows: [qxh qxh qxl  qyh qyh qyl  qzh qzh qzl  1 1]
      (the -|q|^2/2 term is added per-partition at PSUM->SBUF staging)
    """
    c = pos.reshape(2, ng, 3).transpose(0, 2, 1)  # [2, 3, ng]
    hi, lo = _hilo(c)
    t = np.zeros((2, KROWS, ng), np.float16)
    for d in range(3):
        if is_q:
            t[:, 3 * d + 0] = hi[:, d]
            t[:, 3 * d + 1] = hi[:, d]
            t[:, 3 * d + 2] = lo[:, d]
        else:
            t[:, 3 * d + 0] = hi[:, d]
            t[:, 3 * d + 1] = lo[:, d]
            t[:, 3 * d + 2] = hi[:, d]
    if is_q:
        t[:, 9] = 1.0
        t[:, 10] = 1.0
    return np.ascontiguousarray(t)


def shard_inputs(inputs):
    f = lambda name: np.ascontiguousarray(np.asarray(inputs[name], np.float32))
    pos_arrs = {
        "pos": (f("pos"), N3G), "ps2": (f("pos_skip2"), N2G),
        "ps1": (f("pos_skip1"), N1G), "ps0": (f("pos_skip0"), N0G),
    }
    skips = {
        "xs2T": (f("x_skip2"), N2G), "xs1T": (f("x_skip1"), N1G),
        "xs0T": (f("x_skip0"), N0G),
    }
    weights = {k: f(k) for k in ["W3a", "b3a", "W3b", "b3b", "W2a", "b2a",
                                 "W2b", "b2b", "W1a", "b1a", "W1b", "b1b",
                                 "W1c", "b1c"]}
    # (param, source, is_q)
    sides = [("pT3", "pos", False), ("qT3", "ps2", True),
             ("pT2", "ps2", False), ("qT2", "ps1", True),
             ("pT1", "ps1", False), ("qT1", "ps0", True)]
    x_full = f("x")
    in_maps = []
    for c in range(N_CORES):
        m = dict(weights)
        sub_pos = {}
        for nm, (arr, ng) in pos_arrs.items():
            sub = np.ascontiguousarray(arr[2 * c * ng : (2 * c + 2) * ng])
            m[nm] = sub
            sub_pos[nm] = (sub, ng)
        for pnm, src, is_q in sides:
            sub, ng = sub_pos[src]
            m[pnm] = _pack_side(sub, ng, is_q)
        for nm, (arr, ng) in skips.items():
            sub = arr[2 * c * ng : (2 * c + 2) * ng]
            d = sub.shape[1]
            t = sub.reshape(2, ng, d).transpose(0, 2, 1)
            m[nm] = np.ascontiguousarray(t.astype(ml_dtypes.bfloat16))
        m["x"] = np.ascontiguousarray(
            x_full[2 * c * N3G : (2 * c + 2) * N3G].astype(ml_dtypes.bfloat16))
        in_maps.append(m)
    return in_maps


def kernel(**inputs):
    nc = _get_nc()
    in_maps = shard_inputs(inputs)
    from concourse.bass_utils import run_bass_kernel_spmd

    res = run_bass_kernel_spmd(nc, in_maps, list(range(N_CORES)))
    # device writes [g, 3, n]; restore the [n_total, 3] layout
    return np.concatenate(
        [np.asarray(r["out"], np.float32).transpose(0, 2, 1).reshape(-1, 3)
         for r in res.results], axis=0)


if __name__ == "__main__":
    nc, _ = build_module()
    print("build ok")
